# revision 12
# baseline (speedup 1.0000x reference)
"""AI4DEM 5^3-stencil DEM force kernel for 8 TRN2 NeuronCores.

Strategy:
  - Host: scatter particle arrays into dense 96^3 grids (one particle per
    cell), shard along Z into 8 slabs of 12 planes per core.  Each core gets
    a center slab (7 ch x 12 z x 96 x, partition = y) and one extended slab
    in DRAM (100 y_ext rows x 6 ch x 16 z_ext x 100 x_ext) whose halos wrap.
  - Device (SPMD, identical program on 8 cores): engine access patterns
    cannot start at arbitrary partitions, so the y component of each stencil
    shift is realized by DMA-staging a y-rotated copy of the extended slab
    (DMA maps DRAM rows [2-sy, 98-sy) onto partitions 0..96).  Shifts are
    grouped by sy (5 groups); within a group each shift's neighbor view is a
    plain strided view of the staged slab.  For each of the 92 shifts that
    can produce a nonzero contribution, pair forces are computed densely on
    the owned (12,96,96) region and accumulated into 6 force grids.
    The 33 remaining shifts (self + (2,2,1)/(2,2,2) offset families) can
    never produce overlap between real particles; their only effect is the
    reference's "phantom" interaction with empty cells (gathered zeros),
    which depends only on the center particle -> corrected exactly with a
    host-precomputed empty-neighbor count channel.
  - Host: gather the 9 dense output grids at the particle cells.
"""

import numpy as np

G = 96
N = 400000
NCORES = 8
ZP = G // NCORES          # 12 owned z-planes per core
ZE = ZP + 4               # 16 extended z-planes
YE = G + 4                # 100 extended y rows (DRAM only)
XE = G + 4                # 100 extended x
NCH = 7                   # x,y,z,vx,vy,vz,n_empty
OWN = ZP * G              # 1152 free elems per owned (z,x) block
FREE_C = NCH * OWN        # 8064  center slab free size
FREE_S = 6 * ZE * XE      # 9600  extended slab free size
EPS2 = 1e-8               # matches max(eps=1e-4, dist) via sqrt(dist2+eps^2)

_CACHE = {}


def _shift_sets():
    active, dropped = [], []
    for sz in range(-2, 3):
        for sy in range(-2, 3):
            for sx in range(-2, 3):
                if (sz, sy, sx) == (0, 0, 0):
                    continue
                m = sorted((abs(sz), abs(sy), abs(sx)))
                if m in ([1, 2, 2], [2, 2, 2]):
                    dropped.append((sz, sy, sx))
                else:
                    active.append((sz, sy, sx))
    assert len(active) == 92 and len(dropped) == 32
    return active, dropped


def _register_custom_ops():
    """Register the fused DVE ops we need (idempotent)."""
    import concourse.dve_ops as dve_ops_mod
    from concourse.dve_ops import DveOp, OPS, get_dve_sub_opcode, has_src1
    from concourse.dve_spec import Spec, Src0, Src1, sq, lower
    from concourse.dve_uop import DveOpSpec

    def reg(name, spec):
        for op in OPS:
            if op.name == name:
                return op
        tmp = DveOp(name, spec, subdim=False, uops_sha={})
        OPS.append(tmp)
        dve_ops_mod._SUB_OPCODE_FOR_NAME[name] = (
            dve_ops_mod._CUSTOM_DVE_ROW_BASE + len(OPS) - 1
        )
        dve_ops_mod.CUSTOM_DVE_SPECS[name] = spec
        shas = {}
        for ver in ("v3", "v4"):
            try:
                ds = DveOpSpec(
                    name=name,
                    opcode=get_dve_sub_opcode(name),
                    uops=lower(spec, ver=ver),
                    rd1_en=has_src1(spec),
                )
                shas[ver] = ds.sha(ver)
            except Exception:
                pass
        final = DveOp(name, spec, subdim=False, uops_sha=shas)
        for i, op in enumerate(OPS):
            if op.name == name:
                OPS[i] = final
                break
        return final

    sqsum = reg(
        "ANT_SQSUM2",
        Spec(
            body=sq(Src0) + sq(Src1),
            reference=lambda in0, in1, s0, s1, imm2: in0 * in0 + in1 * in1,
        ),
    )
    sqadd = reg(
        "ANT_SQADD",
        Spec(
            body=sq(Src0) + Src1,
            reference=lambda in0, in1, s0, s1, imm2: in0 * in0 + in1,
        ),
    )
    return sqsum, sqadd


def _build(d, kn, eta):
    import concourse.mybir as mybir
    from concourse.bacc import Bacc
    from concourse.tile import TileContext

    SQSUM, SQADD = _register_custom_ops()
    f32 = mybir.dt.float32
    Alu = mybir.AluOpType
    Act = mybir.ActivationFunctionType
    active, _ = _shift_sets()
    by_sy = {sy: [s for s in active if s[1] == sy] for sy in range(-2, 3)}

    nc = Bacc()
    ctr_p = nc.declare_dram_parameter("ctr", [G, FREE_C], f32, isOutput=False)
    ext_p = nc.declare_dram_parameter("ext", [YE, FREE_S], f32, isOutput=False)
    out_p = nc.declare_dram_parameter("out", [G, 9 * OWN], f32, isOutput=True)
    eye_p = nc.declare_dram_parameter("eye", [G, G], f32, isOutput=False)

    with TileContext(nc) as tc:
        with tc.tile_pool(name="persist", bufs=1) as pp:
            C = pp.tile([G, FREE_C], f32, tag="ctr")
            S = pp.tile([G, FREE_S], f32, tag="ext")
            OUTF = pp.tile([G, 6 * OWN], f32, tag="outf")

            for j in range(4):
                w = FREE_C // 4
                nc.sync.dma_start(C[:, j * w : (j + 1) * w], ctr_p[:, j * w : (j + 1) * w])

            def cch(i):  # flat center channel
                return C[:, i * OWN : (i + 1) * OWN]

            CV = C[:, :].rearrange("p (c z x) -> p c z x", c=NCH, z=ZP, x=G)
            SV = S[:, :].rearrange("p (c z x) -> p c z x", c=6, z=ZE, x=XE)

            def nbr(c0, c1, sz, sx):
                return SV[:, c0:c1, 2 - sz : 2 - sz + ZP, 2 - sx : 2 - sx + G]

            # ---- wall forces (channels 6..8), scoped temps, DMA out early
            with tc.tile_pool(name="wall", bufs=1) as wpool:
                W1 = wpool.tile([G, OWN], f32, tag="w1")
                W2 = wpool.tile([G, OWN], f32, tag="w2")
                WO = wpool.tile([G, 3 * OWN], f32, tag="wo")
                WC = wpool.tile([G, 2], f32, tag="wc")
                ds = G * d
                nc.vector.memset(WC[:, 0:1], kn * d)
                nc.vector.memset(WC[:, 1:2], -kn * (ds - 2.0 * d))
                for q in range(3):
                    pv = cch(q)
                    och = WO[:, q * OWN : (q + 1) * OWN]
                    nc.scalar.activation(W1[:, :], pv, Act.Relu, bias=WC[:, 0:1], scale=-kn)
                    nc.vector.scalar_tensor_tensor(
                        W2[:, :], pv, 0.0, W1[:, :], Alu.is_equal, Alu.mult
                    )
                    nc.vector.tensor_sub(och, W1[:, :], W2[:, :])
                    nc.scalar.activation(W1[:, :], pv, Act.Relu, bias=WC[:, 1:2], scale=kn)
                    nc.vector.scalar_tensor_tensor(
                        och, W1[:, :], -1.0, och, Alu.mult, Alu.add
                    )
                for j in range(3):
                    nc.sync.dma_start(
                        out_p[:, (6 + j) * OWN : (7 + j) * OWN],
                        WO[:, j * OWN : (j + 1) * OWN],
                    )

            nc.gpsimd.memset(OUTF[:, :], 0.0)

            with (
                tc.tile_pool(name="work", bufs=1) as wp,
                tc.tile_pool(name="rot", bufs=2) as wr,
                tc.tile_pool(name="psum", bufs=1, space="PSUM") as psp,
            ):
                PSA = psp.tile([G, 3 * OWN], f32, tag="psa")
                EYE = wp.tile([G, G], f32, tag="eye")
                nc.sync.dma_start(EYE[:, :], eye_p[:, :])
                D03 = wp.tile([G, 3 * OWN], f32, tag="d03")
                P6 = wp.tile([G, 6 * OWN], f32, tag="p6")
                M3 = P6  # lifetimes don't overlap: M3 dies before products write P6
                S2A = wp.tile([G, OWN], f32, tag="s2a")
                DIST = wp.tile([G, OWN], f32, tag="dist")
                INV = wp.tile([G, OWN], f32, tag="inv")
                AT = wp.tile([G, OWN], f32, tag="at")
                VNA = wp.tile([G, OWN], f32, tag="vna")
                INV2 = wp.tile([G, OWN], f32, tag="inv2")
                AB = wp.tile([G, 2 * OWN], f32, tag="ab")
                CONST = wp.tile([G, 2], f32, tag="const")
                nc.vector.memset(CONST[:, 0:1], EPS2)
                nc.vector.memset(CONST[:, 1:2], 2.0 * d * kn)

                def v3(t):
                    return t[:, :].rearrange("p (c z x) -> p c z x", c=3, z=ZP, x=G)

                def chan(t, i):
                    return t[:, i * OWN : (i + 1) * OWN]

                def force_block(D36):
                    """Consumes D03/D36, accumulates into OUTF."""
                    # dist2 = dx^2+dy^2+dz^2 (into S2A, in place for the adds).
                    # sq(Src1) in a custom DVE op hits a ~14x slow uop path, so
                    # the first square runs on the idle ScalarE instead.
                    nc.scalar.activation(S2A[:, :], chan(D03, 0), Act.Square)
                    nc.vector._custom_dve(
                        SQADD, out=S2A[:, :], in0=chan(D03, 1), in1=S2A[:, :]
                    )
                    nc.vector._custom_dve(
                        SQADD, out=S2A[:, :], in0=chan(D03, 2), in1=S2A[:, :]
                    )
                    nc.scalar.activation(
                        DIST[:, :], S2A[:, :], Act.Sqrt, bias=CONST[:, 0:1]
                    )
                    nc.vector.reciprocal_approx_fast(out=INV[:, :], in_=DIST[:, :])
                    # AT = kn*relu(2d - dist)  (>0 exactly on the overlap mask)
                    nc.scalar.activation(
                        AT[:, :], DIST[:, :], Act.Relu, bias=CONST[:, 1:2], scale=-kn
                    )
                    # vn numerator
                    nc.vector.tensor_tensor(M3[:, 0 : 3 * OWN], D03[:, :], D36[:, :], Alu.mult)
                    nc.vector.tensor_add(VNA[:, :], chan(M3, 0), chan(M3, 1))
                    nc.vector.tensor_add(VNA[:, :], VNA[:, :], chan(M3, 2))
                    # A = -AT*inv ; B = eta*vn*mask*inv^2
                    nc.vector.scalar_tensor_tensor(
                        AB[:, 0:OWN], AT[:, :], -1.0, INV[:, :], Alu.mult, Alu.mult
                    )
                    nc.scalar.activation(INV2[:, :], INV[:, :], Act.Square)
                    nc.vector.scalar_tensor_tensor(
                        INV2[:, :], AT[:, :], 0.0, INV2[:, :], Alu.is_gt, Alu.mult
                    )
                    nc.vector.scalar_tensor_tensor(
                        AB[:, OWN : 2 * OWN],
                        VNA[:, :],
                        eta,
                        INV2[:, :],
                        Alu.mult,
                        Alu.mult,
                    )

                def products_and_acc(first, last):
                    for h in range(2):
                        a = AB[:, h * OWN : (h + 1) * OWN]
                        for q in range(3):
                            nc.vector.tensor_tensor(
                                chan(P6, 3 * h + q), a, chan(D03, q), Alu.mult
                            )
                    half = 3 * OWN
                    for j in range(7):
                        w0 = j * 512
                        w1 = min(half, w0 + 512)
                        nc.tensor.matmul(
                            PSA[:, w0:w1],
                            EYE[:, :],
                            P6[:, w0:w1],
                            start=first,
                            stop=last,
                        )
                    nc.gpsimd.tensor_add(
                        OUTF[:, half:], OUTF[:, half:], P6[:, half:]
                    )

                first = True
                for sy in (-2, -1, 0, 1, 2):
                    # stage the y-rotated extended slab: S[p] = grid[y = p - sy]
                    for j in range(8):
                        w = FREE_S // 8
                        nc.sync.dma_start(
                            S[:, j * w : (j + 1) * w],
                            ext_p[2 - sy : 2 - sy + G, j * w : (j + 1) * w],
                        )
                    for sz, _sy, sx in by_sy[sy]:
                        D36 = wr.tile([G, 3 * OWN], f32, tag="d36")
                        nc.gpsimd.tensor_tensor(
                            v3(D36), CV[:, 3:6], nbr(3, 6, sz, sx), Alu.subtract
                        )
                        nc.vector.tensor_tensor(
                            v3(D03), CV[:, 0:3], nbr(0, 3, sz, sx), Alu.subtract
                        )
                        force_block(D36)
                        products_and_acc(first, False)
                        first = False

                # phantom correction for the 32 dropped shifts
                D36 = wr.tile([G, 3 * OWN], f32, tag="d36")
                nc.vector.tensor_copy(D03[:, :], C[:, 0 : 3 * OWN])
                nc.gpsimd.tensor_copy(D36[:, :], C[:, 3 * OWN : 6 * OWN])
                force_block(D36)
                nemv = cch(6)
                nc.vector.tensor_tensor(AB[:, 0:OWN], AB[:, 0:OWN], nemv, Alu.mult)
                nc.vector.tensor_tensor(
                    AB[:, OWN : 2 * OWN], AB[:, OWN : 2 * OWN], nemv, Alu.mult
                )
                products_and_acc(False, True)
                nc.vector.tensor_copy(OUTF[:, 0 : 3 * OWN], PSA[:, :])

                for j in range(6):
                    nc.sync.dma_start(
                        out_p[:, j * OWN : (j + 1) * OWN],
                        OUTF[:, j * OWN : (j + 1) * OWN],
                    )

    nc.finalize()
    return nc


def _host_prep(inputs):
    d = float(np.asarray(inputs["d"]))
    x = np.asarray(inputs["compressed_x_grid"], np.float32)
    y = np.asarray(inputs["compressed_y_grid"], np.float32)
    z = np.asarray(inputs["compressed_z_grid"], np.float32)
    vx = np.asarray(inputs["compressed_vx_grid"], np.float32)
    vy = np.asarray(inputs["compressed_vy_grid"], np.float32)
    vz = np.asarray(inputs["compressed_vz_grid"], np.float32)

    cx = np.round(x / np.float32(d)).astype(np.int32)
    cy = np.round(y / np.float32(d)).astype(np.int32)
    cz = np.round(z / np.float32(d)).astype(np.int32)

    grids = np.zeros((NCH, G, G, G), np.float32)
    for i, v in enumerate((x, y, z, vx, vy, vz)):
        grids[i, cz, cy, cx] = v
    occ = np.zeros((G, G, G), np.float32)
    occ[cz, cy, cx] = 1.0

    _, dropped = _shift_sets()
    nocc = np.zeros((G, G, G), np.float32)
    for s in dropped:
        nocc += np.roll(occ, s, axis=(0, 1, 2))
    grids[6] = np.float32(len(dropped)) - nocc

    ys = np.arange(-2, G + 2) % G
    xs = np.arange(-2, G + 2) % G
    in_maps = []
    for k in range(NCORES):
        z0 = k * ZP
        # center slab: (y, ch, z_owned, x_owned)
        ctr = np.ascontiguousarray(
            grids[:, z0 : z0 + ZP].transpose(2, 0, 1, 3)
        ).reshape(G, FREE_C)
        # extended slab: (y_ext, ch, z_ext, x_ext), 6 data channels
        zs = np.arange(z0 - 2, z0 + ZP + 2) % G
        ext = grids[0:6, zs][:, :, ys][:, :, :, xs]  # (6,16,100,100)
        ext = np.ascontiguousarray(ext.transpose(2, 0, 1, 3)).reshape(YE, FREE_S)
        in_maps.append({"ctr": ctr, "ext": ext, "eye": np.eye(G, dtype=np.float32)})
    return in_maps, (cz, cy, cx)


def kernel(**inputs):
    from concourse.bass_utils import run_bass_kernel_spmd

    d = float(np.asarray(inputs["d"]))
    kn = float(np.asarray(inputs["kn"]))
    eta = float(np.asarray(inputs["damping_coefficient_Eta"]))

    in_maps, (cz, cy, cx) = _host_prep(inputs)

    key = (d, kn, eta)
    if key not in _CACHE:
        _CACHE[key] = _build(d, kn, eta)
    nc = _CACHE[key]

    res = run_bass_kernel_spmd(nc, in_maps, core_ids=list(range(NCORES)))
    full = np.empty((9, G, G, G), np.float32)
    for k in range(NCORES):
        o = np.asarray(res.results[k]["out"], np.float32).reshape(G, 9, ZP, G)
        full[:, k * ZP : (k + 1) * ZP] = o.transpose(1, 2, 0, 3)
    return full[:, cz, cy, cx]


# revision 13
# speedup vs baseline: 1.0007x; 1.0007x over previous
"""AI4DEM 5^3-stencil DEM force kernel for 8 TRN2 NeuronCores.

Strategy:
  - Host: scatter particle arrays into dense 96^3 grids (one particle per
    cell), shard along Z into 8 slabs of 12 planes per core.  Each core gets
    a center slab (7 ch x 12 z x 96 x, partition = y) and one extended slab
    in DRAM (100 y_ext rows x 6 ch x 16 z_ext x 100 x_ext) whose halos wrap.
  - Device (SPMD, identical program on 8 cores): engine access patterns
    cannot start at arbitrary partitions, so the y component of each stencil
    shift is realized by DMA-staging a y-rotated copy of the extended slab
    (DMA maps DRAM rows [2-sy, 98-sy) onto partitions 0..96).  Shifts are
    grouped by sy (5 groups); within a group each shift's neighbor view is a
    plain strided view of the staged slab.  For each of the 92 shifts that
    can produce a nonzero contribution, pair forces are computed densely on
    the owned (12,96,96) region and accumulated into 6 force grids.
    The 33 remaining shifts (self + (2,2,1)/(2,2,2) offset families) can
    never produce overlap between real particles; their only effect is the
    reference's "phantom" interaction with empty cells (gathered zeros),
    which depends only on the center particle -> corrected exactly with a
    host-precomputed empty-neighbor count channel.
  - Host: gather the 9 dense output grids at the particle cells.
"""

import numpy as np

G = 96
N = 400000
NCORES = 8
ZP = G // NCORES          # 12 owned z-planes per core
ZE = ZP + 4               # 16 extended z-planes
YE = G + 4                # 100 extended y rows (DRAM only)
XE = G + 4                # 100 extended x
NCH = 7                   # x,y,z,vx,vy,vz,n_empty
OWN = ZP * G              # 1152 free elems per owned (z,x) block
FREE_C = NCH * OWN        # 8064  center slab free size
FREE_S = 6 * ZE * XE      # 9600  extended slab free size
EPS2 = 1e-8               # matches max(eps=1e-4, dist) via sqrt(dist2+eps^2)

_CACHE = {}


def _shift_sets():
    active, dropped = [], []
    for sz in range(-2, 3):
        for sy in range(-2, 3):
            for sx in range(-2, 3):
                if (sz, sy, sx) == (0, 0, 0):
                    continue
                m = sorted((abs(sz), abs(sy), abs(sx)))
                if m in ([1, 2, 2], [2, 2, 2]):
                    dropped.append((sz, sy, sx))
                else:
                    active.append((sz, sy, sx))
    assert len(active) == 92 and len(dropped) == 32
    return active, dropped


def _register_custom_ops():
    """Register the fused DVE ops we need (idempotent)."""
    import concourse.dve_ops as dve_ops_mod
    from concourse.dve_ops import DveOp, OPS, get_dve_sub_opcode, has_src1
    from concourse.dve_spec import Spec, Src0, Src1, sq, lower
    from concourse.dve_uop import DveOpSpec

    def reg(name, spec):
        for op in OPS:
            if op.name == name:
                return op
        tmp = DveOp(name, spec, subdim=False, uops_sha={})
        OPS.append(tmp)
        dve_ops_mod._SUB_OPCODE_FOR_NAME[name] = (
            dve_ops_mod._CUSTOM_DVE_ROW_BASE + len(OPS) - 1
        )
        dve_ops_mod.CUSTOM_DVE_SPECS[name] = spec
        shas = {}
        for ver in ("v3", "v4"):
            try:
                ds = DveOpSpec(
                    name=name,
                    opcode=get_dve_sub_opcode(name),
                    uops=lower(spec, ver=ver),
                    rd1_en=has_src1(spec),
                )
                shas[ver] = ds.sha(ver)
            except Exception:
                pass
        final = DveOp(name, spec, subdim=False, uops_sha=shas)
        for i, op in enumerate(OPS):
            if op.name == name:
                OPS[i] = final
                break
        return final

    sqsum = reg(
        "ANT_SQSUM2",
        Spec(
            body=sq(Src0) + sq(Src1),
            reference=lambda in0, in1, s0, s1, imm2: in0 * in0 + in1 * in1,
        ),
    )
    sqadd = reg(
        "ANT_SQADD",
        Spec(
            body=sq(Src0) + Src1,
            reference=lambda in0, in1, s0, s1, imm2: in0 * in0 + in1,
        ),
    )
    return sqsum, sqadd


def _build(d, kn, eta):
    import concourse.mybir as mybir
    from concourse.bacc import Bacc
    from concourse.tile import TileContext

    SQSUM, SQADD = _register_custom_ops()
    f32 = mybir.dt.float32
    Alu = mybir.AluOpType
    Act = mybir.ActivationFunctionType
    active, _ = _shift_sets()
    by_sy = {sy: [s for s in active if s[1] == sy] for sy in range(-2, 3)}

    nc = Bacc()
    ctr_p = nc.declare_dram_parameter("ctr", [G, FREE_C], f32, isOutput=False)
    ext_p = nc.declare_dram_parameter("ext", [YE, FREE_S], f32, isOutput=False)
    out_p = nc.declare_dram_parameter("out", [G, 9 * OWN], f32, isOutput=True)
    eye_p = nc.declare_dram_parameter("eye", [G, G], f32, isOutput=False)

    with TileContext(nc) as tc:
        with tc.tile_pool(name="persist", bufs=1) as pp:
            C = pp.tile([G, FREE_C], f32, tag="ctr")
            S = pp.tile([G, FREE_S], f32, tag="ext")
            OUTF = pp.tile([G, 6 * OWN], f32, tag="outf")

            for j in range(4):
                w = FREE_C // 4
                nc.sync.dma_start(C[:, j * w : (j + 1) * w], ctr_p[:, j * w : (j + 1) * w])

            def cch(i):  # flat center channel
                return C[:, i * OWN : (i + 1) * OWN]

            CV = C[:, :].rearrange("p (c z x) -> p c z x", c=NCH, z=ZP, x=G)
            SV = S[:, :].rearrange("p (c z x) -> p c z x", c=6, z=ZE, x=XE)

            def nbr(c0, c1, sz, sx):
                return SV[:, c0:c1, 2 - sz : 2 - sz + ZP, 2 - sx : 2 - sx + G]

            # ---- wall forces (channels 6..8), scoped temps, DMA out early
            with tc.tile_pool(name="wall", bufs=1) as wpool:
                W1 = wpool.tile([G, OWN], f32, tag="w1")
                W2 = wpool.tile([G, OWN], f32, tag="w2")
                WO = wpool.tile([G, 3 * OWN], f32, tag="wo")
                WC = wpool.tile([G, 2], f32, tag="wc")
                ds = G * d
                nc.vector.memset(WC[:, 0:1], kn * d)
                nc.vector.memset(WC[:, 1:2], -kn * (ds - 2.0 * d))
                for q in range(3):
                    pv = cch(q)
                    och = WO[:, q * OWN : (q + 1) * OWN]
                    nc.scalar.activation(W1[:, :], pv, Act.Relu, bias=WC[:, 0:1], scale=-kn)
                    nc.vector.scalar_tensor_tensor(
                        W2[:, :], pv, 0.0, W1[:, :], Alu.is_equal, Alu.mult
                    )
                    nc.vector.tensor_sub(och, W1[:, :], W2[:, :])
                    nc.scalar.activation(W1[:, :], pv, Act.Relu, bias=WC[:, 1:2], scale=kn)
                    nc.vector.scalar_tensor_tensor(
                        och, W1[:, :], -1.0, och, Alu.mult, Alu.add
                    )
                for j in range(3):
                    nc.sync.dma_start(
                        out_p[:, (6 + j) * OWN : (7 + j) * OWN],
                        WO[:, j * OWN : (j + 1) * OWN],
                    )

            nc.gpsimd.memset(OUTF[:, :], 0.0)

            with (
                tc.tile_pool(name="work", bufs=1) as wp,
                tc.tile_pool(name="rot", bufs=2) as wr,
                tc.tile_pool(name="psum", bufs=1, space="PSUM") as psp,
            ):
                PSA = psp.tile([G, 3 * OWN], f32, tag="psa")
                EYE = wp.tile([G, G], f32, tag="eye")
                nc.sync.dma_start(EYE[:, :], eye_p[:, :])
                D03 = wp.tile([G, 3 * OWN], f32, tag="d03")
                P6 = wp.tile([G, 6 * OWN], f32, tag="p6")
                M3 = P6  # lifetimes don't overlap: M3 dies before products write P6
                S2A = wp.tile([G, OWN], f32, tag="s2a")
                DIST = wp.tile([G, OWN], f32, tag="dist")
                INV = wp.tile([G, OWN], f32, tag="inv")
                AT = wp.tile([G, OWN], f32, tag="at")
                VNA = wp.tile([G, OWN], f32, tag="vna")
                INV2 = wp.tile([G, OWN], f32, tag="inv2")
                AB = wp.tile([G, 2 * OWN], f32, tag="ab")
                CONST = wp.tile([G, 2], f32, tag="const")
                nc.vector.memset(CONST[:, 0:1], EPS2)
                nc.vector.memset(CONST[:, 1:2], 2.0 * d * kn)

                def v3(t):
                    return t[:, :].rearrange("p (c z x) -> p c z x", c=3, z=ZP, x=G)

                def chan(t, i):
                    return t[:, i * OWN : (i + 1) * OWN]

                def force_block(D36):
                    """Consumes D03/D36, accumulates into OUTF."""
                    # dist2 = dx^2+dy^2+dz^2 (into S2A, in place for the adds).
                    # sq(Src1) in a custom DVE op hits a ~14x slow uop path, so
                    # the first square runs on the idle ScalarE instead.
                    nc.scalar.activation(S2A[:, :], chan(D03, 0), Act.Square)
                    nc.vector._custom_dve(
                        SQADD, out=S2A[:, :], in0=chan(D03, 1), in1=S2A[:, :]
                    )
                    nc.vector._custom_dve(
                        SQADD, out=S2A[:, :], in0=chan(D03, 2), in1=S2A[:, :]
                    )
                    nc.scalar.activation(
                        DIST[:, :], S2A[:, :], Act.Sqrt, bias=CONST[:, 0:1]
                    )
                    nc.vector.reciprocal_approx_fast(out=INV[:, :], in_=DIST[:, :])
                    # AT = kn*relu(2d - dist)  (>0 exactly on the overlap mask)
                    nc.scalar.activation(
                        AT[:, :], DIST[:, :], Act.Relu, bias=CONST[:, 1:2], scale=-kn
                    )
                    # vn numerator
                    nc.vector.tensor_tensor(M3[:, 0 : 3 * OWN], D03[:, :], D36[:, :], Alu.mult)
                    nc.vector.tensor_add(VNA[:, :], chan(M3, 0), chan(M3, 1))
                    nc.vector.tensor_add(VNA[:, :], VNA[:, :], chan(M3, 2))
                    # A = -AT*inv ; B = eta*vn*mask*inv^2
                    nc.vector.scalar_tensor_tensor(
                        AB[:, 0:OWN], AT[:, :], -1.0, INV[:, :], Alu.mult, Alu.mult
                    )
                    nc.scalar.activation(INV2[:, :], INV[:, :], Act.Square)
                    nc.vector.scalar_tensor_tensor(
                        INV2[:, :], AT[:, :], 0.0, INV2[:, :], Alu.is_gt, Alu.mult
                    )
                    nc.vector.scalar_tensor_tensor(
                        AB[:, OWN : 2 * OWN],
                        VNA[:, :],
                        eta,
                        INV2[:, :],
                        Alu.mult,
                        Alu.mult,
                    )

                def products_and_acc(first, last):
                    for h in range(2):
                        a = AB[:, h * OWN : (h + 1) * OWN]
                        for q in range(3):
                            nc.vector.tensor_tensor(
                                chan(P6, 3 * h + q), a, chan(D03, q), Alu.mult
                            )
                    half = 3 * OWN
                    for j in range(7):
                        w0 = j * 512
                        w1 = min(half, w0 + 512)
                        nc.tensor.matmul(
                            PSA[:, w0:w1],
                            EYE[:, :],
                            P6[:, w0:w1],
                            start=first,
                            stop=last,
                        )
                    nc.vector.tensor_add(
                        OUTF[:, half:], OUTF[:, half:], P6[:, half:]
                    )

                first = True
                for sy in (-2, -1, 0, 1, 2):
                    # stage the y-rotated extended slab: S[p] = grid[y = p - sy]
                    for j in range(8):
                        w = FREE_S // 8
                        nc.sync.dma_start(
                            S[:, j * w : (j + 1) * w],
                            ext_p[2 - sy : 2 - sy + G, j * w : (j + 1) * w],
                        )
                    for sz, _sy, sx in by_sy[sy]:
                        D36 = wr.tile([G, 3 * OWN], f32, tag="d36")
                        nc.vector.tensor_tensor(
                            v3(D36), CV[:, 3:6], nbr(3, 6, sz, sx), Alu.subtract
                        )
                        nc.vector.tensor_tensor(
                            v3(D03), CV[:, 0:3], nbr(0, 3, sz, sx), Alu.subtract
                        )
                        force_block(D36)
                        products_and_acc(first, False)
                        first = False

                # phantom correction for the 32 dropped shifts
                D36 = wr.tile([G, 3 * OWN], f32, tag="d36")
                nc.vector.tensor_copy(D03[:, :], C[:, 0 : 3 * OWN])
                nc.vector.tensor_copy(D36[:, :], C[:, 3 * OWN : 6 * OWN])
                force_block(D36)
                nemv = cch(6)
                nc.vector.tensor_tensor(AB[:, 0:OWN], AB[:, 0:OWN], nemv, Alu.mult)
                nc.vector.tensor_tensor(
                    AB[:, OWN : 2 * OWN], AB[:, OWN : 2 * OWN], nemv, Alu.mult
                )
                products_and_acc(False, True)
                nc.vector.tensor_copy(OUTF[:, 0 : 3 * OWN], PSA[:, :])

                for j in range(6):
                    nc.sync.dma_start(
                        out_p[:, j * OWN : (j + 1) * OWN],
                        OUTF[:, j * OWN : (j + 1) * OWN],
                    )

    nc.finalize()
    return nc


def _host_prep(inputs):
    d = float(np.asarray(inputs["d"]))
    x = np.asarray(inputs["compressed_x_grid"], np.float32)
    y = np.asarray(inputs["compressed_y_grid"], np.float32)
    z = np.asarray(inputs["compressed_z_grid"], np.float32)
    vx = np.asarray(inputs["compressed_vx_grid"], np.float32)
    vy = np.asarray(inputs["compressed_vy_grid"], np.float32)
    vz = np.asarray(inputs["compressed_vz_grid"], np.float32)

    cx = np.round(x / np.float32(d)).astype(np.int32)
    cy = np.round(y / np.float32(d)).astype(np.int32)
    cz = np.round(z / np.float32(d)).astype(np.int32)

    grids = np.zeros((NCH, G, G, G), np.float32)
    for i, v in enumerate((x, y, z, vx, vy, vz)):
        grids[i, cz, cy, cx] = v
    occ = np.zeros((G, G, G), np.float32)
    occ[cz, cy, cx] = 1.0

    _, dropped = _shift_sets()
    nocc = np.zeros((G, G, G), np.float32)
    for s in dropped:
        nocc += np.roll(occ, s, axis=(0, 1, 2))
    grids[6] = np.float32(len(dropped)) - nocc

    ys = np.arange(-2, G + 2) % G
    xs = np.arange(-2, G + 2) % G
    in_maps = []
    for k in range(NCORES):
        z0 = k * ZP
        # center slab: (y, ch, z_owned, x_owned)
        ctr = np.ascontiguousarray(
            grids[:, z0 : z0 + ZP].transpose(2, 0, 1, 3)
        ).reshape(G, FREE_C)
        # extended slab: (y_ext, ch, z_ext, x_ext), 6 data channels
        zs = np.arange(z0 - 2, z0 + ZP + 2) % G
        ext = grids[0:6, zs][:, :, ys][:, :, :, xs]  # (6,16,100,100)
        ext = np.ascontiguousarray(ext.transpose(2, 0, 1, 3)).reshape(YE, FREE_S)
        in_maps.append({"ctr": ctr, "ext": ext, "eye": np.eye(G, dtype=np.float32)})
    return in_maps, (cz, cy, cx)


def kernel(**inputs):
    from concourse.bass_utils import run_bass_kernel_spmd

    d = float(np.asarray(inputs["d"]))
    kn = float(np.asarray(inputs["kn"]))
    eta = float(np.asarray(inputs["damping_coefficient_Eta"]))

    in_maps, (cz, cy, cx) = _host_prep(inputs)

    key = (d, kn, eta)
    if key not in _CACHE:
        _CACHE[key] = _build(d, kn, eta)
    nc = _CACHE[key]

    res = run_bass_kernel_spmd(nc, in_maps, core_ids=list(range(NCORES)))
    full = np.empty((9, G, G, G), np.float32)
    for k in range(NCORES):
        o = np.asarray(res.results[k]["out"], np.float32).reshape(G, 9, ZP, G)
        full[:, k * ZP : (k + 1) * ZP] = o.transpose(1, 2, 0, 3)
    return full[:, cz, cy, cx]


# revision 14
# speedup vs baseline: 1.9236x; 1.9222x over previous
"""AI4DEM 5^3-stencil DEM force kernel for 8 TRN2 NeuronCores.

v4: Newton's-third-law pair formulation + TensorEngine accumulation.

  - Host: scatter particles into dense 96^3 grids (one per cell), shard
    along Z into 8 slabs.  Per core DRAM inputs: "ext" (y_ext=100 rows x
    6ch x 16 z_ext x 100 x_ext, halos wrap), "nem" (owned empty-neighbor
    counts for the 32 dropped shifts), "rots" (identity + negated
    y-rotation matrices for the TensorEngine scatter pass).
  - Device (SPMD): engine APs cannot start at arbitrary partitions, so the
    y component of each stencil shift is realized by DMA-staging y-rotated
    slabs (DMA maps DRAM rows onto partitions 0..96).  The 92 active
    shifts are processed as 46 +/-s pairs: the pair's shared quantities
    (diffs, dist, 1/dist, vn, A, B, products P6 = [A,B] x [dx,dy,dz]) are
    computed once on a z/x-extended region; the force field then gets
    P6 at the cell (pass 1, identity weights) and -P6 at the shifted cell
    (pass 2, negated y-rotation weights) via TensorEngine matmuls
    accumulating in PSUM (start=False onto a DVE-zeroed bank region).
    Owned z is processed in two halves of 6 planes so the 6-channel force
    accumulator (6x6x96 fp32) fits in PSUM.
  - The 33 remaining shifts (self + (2,2,1)/(2,2,2) families) can never
    produce overlap between real particles; their only effect is the
    reference's "phantom" interaction with empty cells (gathered zeros),
    corrected exactly with the host-precomputed n_empty channel.
  - Host: gather the 9 dense output grids at the particle cells.
"""

import numpy as np

G = 96
N = 400000
NCORES = 8
ZP = G // NCORES          # 12 owned z-planes per core
HZ = ZP // 2              # 6-plane half slabs
ZE = ZP + 4               # 16 extended z-planes (DRAM)
SZE = 10                  # staged z window per half
YE = G + 4                # 100 extended y rows (DRAM only)
XE = G + 4                # 100 extended x
OWN = ZP * G              # 1152
HOWN = HZ * G             # 576
FREE_S = 6 * SZE * XE     # 6000   staged slab free size
FREE_S0 = 6 * 14 * XE     # 8400   center slab free size (z_ext 2..16)
FREE_E = 6 * ZE * XE      # 9600   DRAM ext row size
RMAX = (HZ + 2) * (G + 2) # 784    max region elems per channel
EPS2 = 1e-8

_CACHE = {}


def _shift_sets():
    active, dropped = [], []
    for sz in range(-2, 3):
        for sy in range(-2, 3):
            for sx in range(-2, 3):
                if (sz, sy, sx) == (0, 0, 0):
                    continue
                m = sorted((abs(sz), abs(sy), abs(sx)))
                if m in ([1, 2, 2], [2, 2, 2]):
                    dropped.append((sz, sy, sx))
                else:
                    active.append((sz, sy, sx))
    assert len(active) == 92 and len(dropped) == 32
    return active, dropped


def _pair_sets():
    """Canonical half of the active shifts: one representative per +/-s pair."""
    active, _ = _shift_sets()
    pairs = [
        s
        for s in active
        if (s[0] > 0)
        or (s[0] == 0 and s[1] > 0)
        or (s[0] == 0 and s[1] == 0 and s[2] > 0)
    ]
    assert len(pairs) == 46
    by_sy = {sy: [p for p in pairs if p[1] == sy] for sy in range(-2, 3)}
    return pairs, by_sy


def _register_custom_ops():
    import concourse.dve_ops as dve_ops_mod
    from concourse.dve_ops import DveOp, OPS, get_dve_sub_opcode, has_src1
    from concourse.dve_spec import Spec, Src0, Src1, sq, lower
    from concourse.dve_uop import DveOpSpec

    def reg(name, spec):
        for op in OPS:
            if op.name == name:
                return op
        tmp = DveOp(name, spec, subdim=False, uops_sha={})
        OPS.append(tmp)
        dve_ops_mod._SUB_OPCODE_FOR_NAME[name] = (
            dve_ops_mod._CUSTOM_DVE_ROW_BASE + len(OPS) - 1
        )
        dve_ops_mod.CUSTOM_DVE_SPECS[name] = spec
        shas = {}
        for ver in ("v3", "v4"):
            try:
                ds = DveOpSpec(
                    name=name,
                    opcode=get_dve_sub_opcode(name),
                    uops=lower(spec, ver=ver),
                    rd1_en=has_src1(spec),
                )
                shas[ver] = ds.sha(ver)
            except Exception:
                pass
        final = DveOp(name, spec, subdim=False, uops_sha=shas)
        for i, op in enumerate(OPS):
            if op.name == name:
                OPS[i] = final
                break
        return final

    sqadd = reg(
        "ANT_SQADD",
        Spec(
            body=sq(Src0) + Src1,
            reference=lambda in0, in1, s0, s1, imm2: in0 * in0 + in1,
        ),
    )
    return sqadd


def _build(d, kn, eta):
    import concourse.mybir as mybir
    from concourse.bacc import Bacc
    from concourse.tile import TileContext

    SQADD = _register_custom_ops()
    f32 = mybir.dt.float32
    Alu = mybir.AluOpType
    Act = mybir.ActivationFunctionType
    pairs, by_sy = _pair_sets()

    # PSUM force-accumulator chunks (bank crossing verified OK on HW)
    chunks = []
    for ch in range(6):
        base = ch * HOWN
        chunks.append((base, base + 5 * G, ch, 0, 5, 0, G))
        chunks.append((base + 5 * G, base + HOWN, ch, 5, 6, 0, G))

    nc = Bacc()
    ext_p = nc.declare_dram_parameter("ext", [YE, FREE_E], f32, isOutput=False)
    nem_p = nc.declare_dram_parameter("nem", [G, OWN], f32, isOutput=False)
    rot_p = nc.declare_dram_parameter("rots", [G, 6 * G], f32, isOutput=False)
    out_p = nc.declare_dram_parameter("out", [G, 9 * OWN], f32, isOutput=True)

    with TileContext(nc) as tc:
        with tc.tile_pool(name="persist", bufs=1) as pp:
            S0 = pp.tile([G, FREE_S0], f32, tag="s0")
            NEM = pp.tile([G, OWN], f32, tag="nem")
            ROTS = pp.tile([G, 6 * G], f32, tag="rots")
            OUTF = pp.tile([G, 3456], f32, tag="outf")

            # center slab: ext rows [2, 98), z_ext [2, 16)
            for c in range(6):
                nc.sync.dma_start(
                    S0[:, c * 14 * XE : (c + 1) * 14 * XE],
                    ext_p[2 : 2 + G, c * ZE * XE + 2 * XE : c * ZE * XE + 16 * XE],
                )
            nc.sync.dma_start(NEM[:, :], nem_p[:, :])
            nc.sync.dma_start(ROTS[:, :], rot_p[:, :])

            SV0 = S0[:, :].rearrange("p (c z x) -> p c z x", c=6, z=14, x=XE)

            def rot(i):  # 0 = +I ; 1.. = -rot(sy=-2..2)
                return ROTS[:, i * G : (i + 1) * G]

            # ---- wall forces -> out channels 6..8 (computed once)
            with tc.tile_pool(name="wall", bufs=1) as wpool:
                W1 = wpool.tile([G, OWN], f32, tag="w1")
                W2 = wpool.tile([G, OWN], f32, tag="w2")
                WO = wpool.tile([G, 3 * OWN], f32, tag="wo")
                WC = wpool.tile([G, 2], f32, tag="wc")
                dsz = G * d
                nc.vector.memset(WC[:, 0:1], kn * d)
                nc.vector.memset(WC[:, 1:2], -kn * (dsz - 2.0 * d))

                def vzx(ap):
                    return ap.rearrange("p (z x) -> p z x", z=ZP, x=G)

                for q in range(3):
                    pv = SV0[:, q, 0:ZP, 2 : 2 + G]
                    och = vzx(WO[:, q * OWN : (q + 1) * OWN])
                    nc.scalar.activation(
                        vzx(W1[:, :]), pv, Act.Relu, bias=WC[:, 0:1], scale=-kn
                    )
                    nc.vector.scalar_tensor_tensor(
                        vzx(W2[:, :]), pv, 0.0, vzx(W1[:, :]), Alu.is_equal, Alu.mult
                    )
                    nc.vector.tensor_sub(och, vzx(W1[:, :]), vzx(W2[:, :]))
                    nc.scalar.activation(
                        vzx(W1[:, :]), pv, Act.Relu, bias=WC[:, 1:2], scale=kn
                    )
                    nc.vector.scalar_tensor_tensor(
                        och, vzx(W1[:, :]), -1.0, och, Alu.mult, Alu.add
                    )
                for j in range(3):
                    nc.sync.dma_start(
                        out_p[:, (6 + j) * OWN : (7 + j) * OWN],
                        WO[:, j * OWN : (j + 1) * OWN],
                    )

            with (
                tc.tile_pool(name="work", bufs=1) as wp,
                tc.tile_pool(name="stage", bufs=2) as sp,
                tc.tile_pool(name="psum", bufs=1, space="PSUM") as psp,
            ):
                PSA = psp.tile([G, 3456], f32, tag="psa")
                D03 = wp.tile([G, 3 * RMAX], f32, tag="d03")
                D36 = wp.tile([G, 3 * RMAX], f32, tag="d36")
                P6 = wp.tile([G, 6 * RMAX], f32, tag="p6")
                S2A = wp.tile([G, RMAX], f32, tag="s2a")
                DIST = wp.tile([G, RMAX], f32, tag="dist")
                INV = wp.tile([G, RMAX], f32, tag="inv")
                AT = wp.tile([G, RMAX], f32, tag="at")
                VNA = wp.tile([G, RMAX], f32, tag="vna")
                INV2 = wp.tile([G, RMAX], f32, tag="inv2")
                AB = wp.tile([G, 2 * RMAX], f32, tag="ab")
                CONST = wp.tile([G, 2], f32, tag="const")
                nc.vector.memset(CONST[:, 0:1], EPS2)
                nc.vector.memset(CONST[:, 1:2], 2.0 * d * kn)

                def force_core(d03v, d36v, zr, xr, ab_scale_ap):
                    """Given diff views (3,zr,xr), produce A (AB[0]), B (AB[1])
                    and the products P6 = [A,B] x [dx,dy,dz] on the region."""
                    fr = zr * xr

                    def t2(t):
                        return t[:, 0:fr].rearrange("p (z x) -> p z x", z=zr, x=xr)

                    nc.scalar.activation(t2(S2A), d03v[:, 0], Act.Square)
                    nc.vector._custom_dve(
                        SQADD, out=t2(S2A), in0=d03v[:, 1], in1=t2(S2A)
                    )
                    nc.vector._custom_dve(
                        SQADD, out=t2(S2A), in0=d03v[:, 2], in1=t2(S2A)
                    )
                    nc.scalar.activation(
                        DIST[:, 0:fr], S2A[:, 0:fr], Act.Sqrt, bias=CONST[:, 0:1]
                    )
                    nc.vector.reciprocal_approx_fast(
                        out=INV[:, 0:fr], in_=DIST[:, 0:fr]
                    )
                    nc.scalar.activation(
                        AT[:, 0:fr], DIST[:, 0:fr], Act.Relu,
                        bias=CONST[:, 1:2], scale=-kn,
                    )
                    # vn numerator (M3 scratch aliased into P6)
                    nc.vector.tensor_tensor(
                        P6[:, 0 : 3 * fr], D03[:, 0 : 3 * fr], D36[:, 0 : 3 * fr],
                        Alu.mult,
                    )
                    nc.vector.tensor_add(
                        VNA[:, 0:fr], P6[:, 0:fr], P6[:, fr : 2 * fr]
                    )
                    nc.vector.tensor_add(
                        VNA[:, 0:fr], VNA[:, 0:fr], P6[:, 2 * fr : 3 * fr]
                    )
                    # A = -AT*inv ; B = eta*vn*mask*inv^2
                    nc.vector.scalar_tensor_tensor(
                        AB[:, 0:fr], AT[:, 0:fr], -1.0, INV[:, 0:fr],
                        Alu.mult, Alu.mult,
                    )
                    nc.scalar.activation(INV2[:, 0:fr], INV[:, 0:fr], Act.Square)
                    nc.vector.scalar_tensor_tensor(
                        INV2[:, 0:fr], AT[:, 0:fr], 0.0, INV2[:, 0:fr],
                        Alu.is_gt, Alu.mult,
                    )
                    nc.vector.scalar_tensor_tensor(
                        AB[:, RMAX : RMAX + fr], VNA[:, 0:fr], eta, INV2[:, 0:fr],
                        Alu.mult, Alu.mult,
                    )
                    if ab_scale_ap is not None:
                        nc.vector.tensor_tensor(
                            AB[:, 0:fr], AB[:, 0:fr], ab_scale_ap, Alu.mult
                        )
                        nc.vector.tensor_tensor(
                            AB[:, RMAX : RMAX + fr], AB[:, RMAX : RMAX + fr],
                            ab_scale_ap, Alu.mult,
                        )
                    for hg in range(2):
                        a = AB[:, hg * RMAX : hg * RMAX + fr]
                        for q in range(3):
                            nc.vector.tensor_tensor(
                                P6[:, (3 * hg + q) * fr : (3 * hg + q + 1) * fr],
                                a,
                                D03[:, q * fr : (q + 1) * fr],
                                Alu.mult,
                            )
                    return fr

                def pe_pass(rot_idx, zoff, xoff, zr, xr, fr, stop):
                    """PSA[(ch, z, x)] += rot . P6[(ch, z+zoff, x+xoff)]"""
                    P6v = P6[:, 0 : 6 * fr].rearrange(
                        "p (c z x) -> p c z x", c=6, z=zr, x=xr
                    )
                    for k, (o0, o1, ch, z0, z1, x0, x1) in enumerate(chunks):
                        nc.tensor.matmul(
                            PSA[:, o0:o1],
                            rot(rot_idx),
                            P6v[:, ch, z0 + zoff : z1 + zoff, x0 + xoff : x1 + xoff],
                            start=False,
                            stop=stop and k == len(chunks) - 1,
                            skip_group_check=True,
                        )

                for h in range(2):
                    nc.vector.memset(PSA[:, :], 0.0)
                    for sy in (-2, -1, 0, 1, 2):
                        if not by_sy[sy]:
                            continue
                        S = sp.tile([G, FREE_S], f32, tag="sst")
                        # staged neighbor slab: rows y = p - sy, z_ext window
                        # [6h, 6h+10) of the DRAM ext slab
                        for c in range(6):
                            nc.sync.dma_start(
                                S[:, c * SZE * XE : (c + 1) * SZE * XE],
                                ext_p[
                                    2 - sy : 2 - sy + G,
                                    c * ZE * XE
                                    + 6 * h * XE : c * ZE * XE
                                    + (6 * h + SZE) * XE,
                                ],
                            )
                        SV = S[:, :].rearrange(
                            "p (c z x) -> p c z x", c=6, z=SZE, x=XE
                        )
                        for sz, _sy, sx in by_sy[sy]:
                            zr = HZ + sz
                            xr = G + abs(sx)
                            xlo = min(sx, 0)
                            fr = zr * xr

                            def dreg(t):
                                return t[:, 0 : 3 * fr].rearrange(
                                    "p (c z x) -> p c z x", c=3, z=zr, x=xr
                                )

                            nc.vector.tensor_tensor(
                                dreg(D03),
                                SV0[:, 0:3, 6 * h : 6 * h + zr,
                                    2 + xlo : 2 + xlo + xr],
                                SV[:, 0:3, 2 - sz : 2 - sz + zr,
                                   2 + xlo - sx : 2 + xlo - sx + xr],
                                Alu.subtract,
                            )
                            nc.vector.tensor_tensor(
                                dreg(D36),
                                SV0[:, 3:6, 6 * h : 6 * h + zr,
                                    2 + xlo : 2 + xlo + xr],
                                SV[:, 3:6, 2 - sz : 2 - sz + zr,
                                   2 + xlo - sx : 2 + xlo - sx + xr],
                                Alu.subtract,
                            )
                            force_core(dreg(D03), dreg(D36), zr, xr, None)
                            pe_pass(0, 0, -xlo, zr, xr, fr, False)
                            pe_pass(1 + (sy + 2), sz, sx - xlo, zr, xr, fr, False)

                    # phantom correction for the 32 dropped shifts (this half)
                    fr = HOWN
                    nemv = NEM[:, h * HOWN : (h + 1) * HOWN]

                    def dregp(t):
                        return t[:, 0 : 3 * fr].rearrange(
                            "p (c z x) -> p c z x", c=3, z=HZ, x=G
                        )

                    nc.vector.tensor_copy(
                        dregp(D03), SV0[:, 0:3, 6 * h : 6 * h + HZ, 2 : 2 + G]
                    )
                    nc.vector.tensor_copy(
                        dregp(D36), SV0[:, 3:6, 6 * h : 6 * h + HZ, 2 : 2 + G]
                    )
                    force_core(dregp(D03), dregp(D36), HZ, G, nemv)
                    pe_pass(0, 0, 0, HZ, G, fr, True)

                    # evacuate PSUM -> SBUF -> DRAM (channel c, half h)
                    nc.vector.tensor_copy(OUTF[:, :], PSA[:, :])
                    for c in range(6):
                        nc.sync.dma_start(
                            out_p[:, c * OWN + h * HOWN : c * OWN + (h + 1) * HOWN],
                            OUTF[:, c * HOWN : (c + 1) * HOWN],
                        )

    nc.finalize()
    return nc


def _host_prep(inputs):
    d = float(np.asarray(inputs["d"]))
    x = np.asarray(inputs["compressed_x_grid"], np.float32)
    y = np.asarray(inputs["compressed_y_grid"], np.float32)
    z = np.asarray(inputs["compressed_z_grid"], np.float32)
    vx = np.asarray(inputs["compressed_vx_grid"], np.float32)
    vy = np.asarray(inputs["compressed_vy_grid"], np.float32)
    vz = np.asarray(inputs["compressed_vz_grid"], np.float32)

    cx = np.round(x / np.float32(d)).astype(np.int32)
    cy = np.round(y / np.float32(d)).astype(np.int32)
    cz = np.round(z / np.float32(d)).astype(np.int32)

    grids = np.zeros((6, G, G, G), np.float32)
    for i, v in enumerate((x, y, z, vx, vy, vz)):
        grids[i, cz, cy, cx] = v
    occ = np.zeros((G, G, G), np.float32)
    occ[cz, cy, cx] = 1.0

    _, dropped = _shift_sets()
    nocc = np.zeros((G, G, G), np.float32)
    for s in dropped:
        nocc += np.roll(occ, s, axis=(0, 1, 2))
    nem = np.float32(len(dropped)) - nocc

    rots = np.zeros((G, 6 * G), np.float32)
    rots[np.arange(G), np.arange(G)] = 1.0  # +I
    for i, sy in enumerate((-2, -1, 0, 1, 2)):
        rots[(np.arange(G) + sy) % G, (i + 1) * G + np.arange(G)] = -1.0

    ys = np.arange(-2, G + 2) % G
    xs = np.arange(-2, G + 2) % G
    in_maps = []
    for k in range(NCORES):
        z0 = k * ZP
        zs = np.arange(z0 - 2, z0 + ZP + 2) % G
        ext = grids[:, zs][:, :, ys][:, :, :, xs]  # (6,16,100,100)
        ext = np.ascontiguousarray(ext.transpose(2, 0, 1, 3)).reshape(YE, FREE_E)
        nemk = np.ascontiguousarray(
            nem[z0 : z0 + ZP].transpose(1, 0, 2)
        ).reshape(G, OWN)
        in_maps.append({"ext": ext, "nem": nemk, "rots": rots})
    return in_maps, (cz, cy, cx)


def kernel(**inputs):
    from concourse.bass_utils import run_bass_kernel_spmd

    d = float(np.asarray(inputs["d"]))
    kn = float(np.asarray(inputs["kn"]))
    eta = float(np.asarray(inputs["damping_coefficient_Eta"]))

    in_maps, (cz, cy, cx) = _host_prep(inputs)

    key = (d, kn, eta)
    if key not in _CACHE:
        _CACHE[key] = _build(d, kn, eta)
    nc = _CACHE[key]

    res = run_bass_kernel_spmd(nc, in_maps, core_ids=list(range(NCORES)))
    full = np.empty((9, G, G, G), np.float32)
    for k in range(NCORES):
        o = np.asarray(res.results[k]["out"], np.float32).reshape(G, 9, ZP, G)
        full[:, k * ZP : (k + 1) * ZP] = o.transpose(1, 2, 0, 3)
    return full[:, cz, cy, cx]


# revision 16
# speedup vs baseline: 2.1339x; 1.1094x over previous
"""AI4DEM 5^3-stencil DEM force kernel for 8 TRN2 NeuronCores.

v4: Newton's-third-law pair formulation + TensorEngine accumulation.

  - Host: scatter particles into dense 96^3 grids (one per cell), shard
    along Z into 8 slabs.  Per core DRAM inputs: "ext" (y_ext=100 rows x
    6ch x 16 z_ext x 100 x_ext, halos wrap), "nem" (owned empty-neighbor
    counts for the 32 dropped shifts), "rots" (identity + negated
    y-rotation matrices for the TensorEngine scatter pass).
  - Device (SPMD): engine APs cannot start at arbitrary partitions, so the
    y component of each stencil shift is realized by DMA-staging y-rotated
    slabs (DMA maps DRAM rows onto partitions 0..96).  The 92 active
    shifts are processed as 46 +/-s pairs: the pair's shared quantities
    (diffs, dist, 1/dist, vn, A, B, products P6 = [A,B] x [dx,dy,dz]) are
    computed once on a z/x-extended region; the force field then gets
    P6 at the cell (pass 1, identity weights) and -P6 at the shifted cell
    (pass 2, negated y-rotation weights) via TensorEngine matmuls
    accumulating in PSUM (start=False onto a DVE-zeroed bank region).
    Owned z is processed in two halves of 6 planes so the 6-channel force
    accumulator (6x6x96 fp32) fits in PSUM.
  - The 33 remaining shifts (self + (2,2,1)/(2,2,2) families) can never
    produce overlap between real particles; their only effect is the
    reference's "phantom" interaction with empty cells (gathered zeros),
    corrected exactly with the host-precomputed n_empty channel.
  - Host: gather the 9 dense output grids at the particle cells.
"""

import numpy as np

G = 96
N = 400000
NCORES = 8
ZP = G // NCORES          # 12 owned z-planes per core
HZ = ZP // 2              # 6-plane half slabs
ZE = ZP + 4               # 16 extended z-planes (DRAM)
SZE = 10                  # staged z window per half
YE = G + 4                # 100 extended y rows (DRAM only)
XE = G + 4                # 100 extended x
OWN = ZP * G              # 1152
HOWN = HZ * G             # 576
FREE_S = 6 * SZE * XE     # 6000   staged slab free size
FREE_S0 = 6 * 14 * XE     # 8400   center slab free size (z_ext 2..16)
FREE_E = 6 * ZE * XE      # 9600   DRAM ext row size
RMAX = (HZ + 2) * (G + 2) # 784    max region elems per channel
EPS2 = 1e-8

_CACHE = {}


def _shift_sets():
    active, dropped = [], []
    for sz in range(-2, 3):
        for sy in range(-2, 3):
            for sx in range(-2, 3):
                if (sz, sy, sx) == (0, 0, 0):
                    continue
                m = sorted((abs(sz), abs(sy), abs(sx)))
                if m in ([1, 2, 2], [2, 2, 2]):
                    dropped.append((sz, sy, sx))
                else:
                    active.append((sz, sy, sx))
    assert len(active) == 92 and len(dropped) == 32
    return active, dropped


def _pair_sets():
    """Canonical half of the active shifts: one representative per +/-s pair."""
    active, _ = _shift_sets()
    pairs = [
        s
        for s in active
        if (s[0] > 0)
        or (s[0] == 0 and s[1] > 0)
        or (s[0] == 0 and s[1] == 0 and s[2] > 0)
    ]
    assert len(pairs) == 46
    by_sy = {sy: [p for p in pairs if p[1] == sy] for sy in range(-2, 3)}
    return pairs, by_sy


def _register_custom_ops():
    import concourse.dve_ops as dve_ops_mod
    from concourse.dve_ops import DveOp, OPS, get_dve_sub_opcode, has_src1
    from concourse.dve_spec import Spec, Src0, Src1, sq, lower
    from concourse.dve_uop import DveOpSpec

    def reg(name, spec):
        for op in OPS:
            if op.name == name:
                return op
        tmp = DveOp(name, spec, subdim=False, uops_sha={})
        OPS.append(tmp)
        dve_ops_mod._SUB_OPCODE_FOR_NAME[name] = (
            dve_ops_mod._CUSTOM_DVE_ROW_BASE + len(OPS) - 1
        )
        dve_ops_mod.CUSTOM_DVE_SPECS[name] = spec
        shas = {}
        for ver in ("v3", "v4"):
            try:
                ds = DveOpSpec(
                    name=name,
                    opcode=get_dve_sub_opcode(name),
                    uops=lower(spec, ver=ver),
                    rd1_en=has_src1(spec),
                )
                shas[ver] = ds.sha(ver)
            except Exception:
                pass
        final = DveOp(name, spec, subdim=False, uops_sha=shas)
        for i, op in enumerate(OPS):
            if op.name == name:
                OPS[i] = final
                break
        return final

    sqadd = reg(
        "ANT_SQADD",
        Spec(
            body=sq(Src0) + Src1,
            reference=lambda in0, in1, s0, s1, imm2: in0 * in0 + in1,
        ),
    )
    return sqadd


def _build(d, kn, eta):
    import concourse.mybir as mybir
    from concourse.bacc import Bacc
    from concourse.tile import TileContext

    SQADD = _register_custom_ops()
    f32 = mybir.dt.float32
    Alu = mybir.AluOpType
    Act = mybir.ActivationFunctionType
    pairs, by_sy = _pair_sets()

    # PSUM force-accumulator chunks (bank crossing verified OK on HW)
    chunks = []
    for ch in range(6):
        base = ch * HOWN
        chunks.append((base, base + 5 * G, ch, 0, 5, 0, G))
        chunks.append((base + 5 * G, base + HOWN, ch, 5, 6, 0, G))

    nc = Bacc()
    ext_p = nc.declare_dram_parameter("ext", [YE, FREE_E], f32, isOutput=False)
    nem_p = nc.declare_dram_parameter("nem", [G, OWN], f32, isOutput=False)
    rot_p = nc.declare_dram_parameter("rots", [G, 6 * G], f32, isOutput=False)
    out_p = nc.declare_dram_parameter("out", [G, 9 * OWN], f32, isOutput=True)

    with TileContext(nc) as tc:
        with tc.tile_pool(name="persist", bufs=1) as pp:
            S0 = pp.tile([G, FREE_S0], f32, tag="s0")
            NEM = pp.tile([G, OWN], f32, tag="nem")
            ROTS = pp.tile([G, 6 * G], f32, tag="rots")
            OUTF = pp.tile([G, 3456], f32, tag="outf")

            # center slab: ext rows [2, 98), z_ext [2, 16)
            for c in range(6):
                nc.sync.dma_start(
                    S0[:, c * 14 * XE : (c + 1) * 14 * XE],
                    ext_p[2 : 2 + G, c * ZE * XE + 2 * XE : c * ZE * XE + 16 * XE],
                )
            nc.sync.dma_start(NEM[:, :], nem_p[:, :])
            nc.sync.dma_start(ROTS[:, :], rot_p[:, :])

            SV0 = S0[:, :].rearrange("p (c z x) -> p c z x", c=6, z=14, x=XE)

            def rot(i):  # 0 = +I ; 1.. = -rot(sy=-2..2)
                return ROTS[:, i * G : (i + 1) * G]

            # ---- wall forces -> out channels 6..8 (computed once)
            with tc.tile_pool(name="wall", bufs=1) as wpool:
                W1 = wpool.tile([G, OWN], f32, tag="w1")
                W2 = wpool.tile([G, OWN], f32, tag="w2")
                WO = wpool.tile([G, 3 * OWN], f32, tag="wo")
                WC = wpool.tile([G, 2], f32, tag="wc")
                dsz = G * d
                nc.vector.memset(WC[:, 0:1], kn * d)
                nc.vector.memset(WC[:, 1:2], -kn * (dsz - 2.0 * d))

                def vzx(ap):
                    return ap.rearrange("p (z x) -> p z x", z=ZP, x=G)

                for q in range(3):
                    pv = SV0[:, q, 0:ZP, 2 : 2 + G]
                    och = vzx(WO[:, q * OWN : (q + 1) * OWN])
                    nc.scalar.activation(
                        vzx(W1[:, :]), pv, Act.Relu, bias=WC[:, 0:1], scale=-kn
                    )
                    nc.vector.scalar_tensor_tensor(
                        vzx(W2[:, :]), pv, 0.0, vzx(W1[:, :]), Alu.is_equal, Alu.mult
                    )
                    nc.vector.tensor_sub(och, vzx(W1[:, :]), vzx(W2[:, :]))
                    nc.scalar.activation(
                        vzx(W1[:, :]), pv, Act.Relu, bias=WC[:, 1:2], scale=kn
                    )
                    nc.vector.scalar_tensor_tensor(
                        och, vzx(W1[:, :]), -1.0, och, Alu.mult, Alu.add
                    )
                for j in range(3):
                    nc.sync.dma_start(
                        out_p[:, (6 + j) * OWN : (7 + j) * OWN],
                        WO[:, j * OWN : (j + 1) * OWN],
                    )

            with (
                tc.tile_pool(name="work", bufs=1) as wp,
                tc.tile_pool(name="stage", bufs=2) as sp,
                tc.tile_pool(name="p6pool", bufs=2) as p6p,
                tc.tile_pool(name="psum", bufs=1, space="PSUM") as psp,
            ):
                PSA = psp.tile([G, 3456], f32, tag="psa")
                D6 = wp.tile([G, 6 * RMAX], f32, tag="d6")
                M3 = wp.tile([G, 3 * RMAX], f32, tag="m3")
                S2A = wp.tile([G, RMAX], f32, tag="s2a")
                DIST = wp.tile([G, RMAX], f32, tag="dist")
                INV = wp.tile([G, RMAX], f32, tag="inv")
                AT = wp.tile([G, RMAX], f32, tag="at")
                VNA = wp.tile([G, RMAX], f32, tag="vna")
                INV2 = wp.tile([G, RMAX], f32, tag="inv2")
                AB = wp.tile([G, 2 * RMAX], f32, tag="ab")
                CONST = wp.tile([G, 2], f32, tag="const")

                nc.vector.memset(CONST[:, 0:1], EPS2)
                nc.vector.memset(CONST[:, 1:2], 2.0 * d * kn)

                def force_core(P6, d03v, d36v, zr, xr, ab_scale_ap):
                    """Given diff views (3,zr,xr) living in D6, produce A, B
                    and the products P6 = [A,B] x [dx,dy,dz] on the region."""
                    fr = zr * xr

                    def t2(t):
                        return t[:, 0:fr].rearrange("p (z x) -> p z x", z=zr, x=xr)

                    nc.scalar.activation(t2(S2A), d03v[:, 0], Act.Square)
                    # vn numerator first: fills the ACT-latency bubble on DVE
                    nc.vector.tensor_tensor(
                        M3[:, 0 : 3 * fr], D6[:, 0 : 3 * fr],
                        D6[:, 3 * RMAX : 3 * RMAX + 3 * fr], Alu.mult,
                    )
                    nc.vector.tensor_add(
                        VNA[:, 0:fr], M3[:, 0:fr], M3[:, fr : 2 * fr]
                    )
                    nc.vector.tensor_add(
                        VNA[:, 0:fr], VNA[:, 0:fr], M3[:, 2 * fr : 3 * fr]
                    )
                    nc.vector._custom_dve(
                        SQADD, out=t2(S2A), in0=d03v[:, 1], in1=t2(S2A)
                    )
                    nc.vector._custom_dve(
                        SQADD, out=t2(S2A), in0=d03v[:, 2], in1=t2(S2A)
                    )
                    nc.scalar.activation(
                        DIST[:, 0:fr], S2A[:, 0:fr], Act.Sqrt, bias=CONST[:, 0:1]
                    )
                    nc.vector.reciprocal_approx_fast(
                        out=INV[:, 0:fr], in_=DIST[:, 0:fr]
                    )
                    nc.scalar.activation(
                        AT[:, 0:fr], DIST[:, 0:fr], Act.Relu,
                        bias=CONST[:, 1:2], scale=-kn,
                    )
                    # A = -AT*inv ; B = eta*vn*mask*inv^2
                    nc.vector.scalar_tensor_tensor(
                        AB[:, 0:fr], AT[:, 0:fr], -1.0, INV[:, 0:fr],
                        Alu.mult, Alu.mult,
                    )
                    nc.scalar.activation(INV2[:, 0:fr], INV[:, 0:fr], Act.Square)
                    nc.vector.scalar_tensor_tensor(
                        INV2[:, 0:fr], AT[:, 0:fr], 0.0, INV2[:, 0:fr],
                        Alu.is_gt, Alu.mult,
                    )
                    nc.vector.scalar_tensor_tensor(
                        AB[:, RMAX : RMAX + fr], VNA[:, 0:fr], eta, INV2[:, 0:fr],
                        Alu.mult, Alu.mult,
                    )
                    if ab_scale_ap is not None:
                        nc.vector.tensor_tensor(
                            AB[:, 0:fr], AB[:, 0:fr], ab_scale_ap, Alu.mult
                        )
                        nc.vector.tensor_tensor(
                            AB[:, RMAX : RMAX + fr], AB[:, RMAX : RMAX + fr],
                            ab_scale_ap, Alu.mult,
                        )
                    for hg in range(2):
                        a = AB[:, hg * RMAX : hg * RMAX + fr]
                        for q in range(3):
                            nc.vector.tensor_tensor(
                                P6[:, (3 * hg + q) * fr : (3 * hg + q + 1) * fr],
                                a,
                                D6[:, q * fr : (q + 1) * fr],
                                Alu.mult,
                            )
                    return fr

                def pe_pass(P6, rot_idx, zoff, xoff, zr, xr, fr, stop):
                    """PSA[(ch, z, x)] += rot . P6[(ch, z+zoff, x+xoff)]"""
                    P6v = P6[:, 0 : 6 * fr].rearrange(
                        "p (c z x) -> p c z x", c=6, z=zr, x=xr
                    )
                    for k, (o0, o1, ch, z0, z1, x0, x1) in enumerate(chunks):
                        nc.tensor.matmul(
                            PSA[:, o0:o1],
                            rot(rot_idx),
                            P6v[:, ch, z0 + zoff : z1 + zoff, x0 + xoff : x1 + xoff],
                            start=False,
                            stop=stop and k == len(chunks) - 1,
                            skip_group_check=True,
                        )

                for h in range(2):
                    nc.vector.memset(PSA[:, :], 0.0)
                    for sy in (-2, -1, 0, 1, 2):
                        if not by_sy[sy]:
                            continue
                        S = sp.tile([G, FREE_S], f32, tag="sst")
                        # staged neighbor slab: rows y = p - sy, z_ext window
                        # [6h, 6h+10) of the DRAM ext slab
                        for c in range(6):
                            nc.sync.dma_start(
                                S[:, c * SZE * XE : (c + 1) * SZE * XE],
                                ext_p[
                                    2 - sy : 2 - sy + G,
                                    c * ZE * XE
                                    + 6 * h * XE : c * ZE * XE
                                    + (6 * h + SZE) * XE,
                                ],
                            )
                        SV = S[:, :].rearrange(
                            "p (c z x) -> p c z x", c=6, z=SZE, x=XE
                        )
                        for sz, _sy, sx in by_sy[sy]:
                            zr = HZ + sz
                            xr = G + abs(sx)
                            xlo = min(sx, 0)
                            fr = zr * xr
                            P6 = p6p.tile([G, 6 * RMAX], f32, tag="p6")

                            nc.vector.tensor_tensor(
                                D6[:, 0 : 3 * fr].rearrange(
                                    "p (c z x) -> p c z x", c=3, z=zr, x=xr
                                ),
                                SV0[:, 0:3, 6 * h : 6 * h + zr,
                                    2 + xlo : 2 + xlo + xr],
                                SV[:, 0:3, 2 - sz : 2 - sz + zr,
                                   2 + xlo - sx : 2 + xlo - sx + xr],
                                Alu.subtract,
                            )
                            nc.vector.tensor_tensor(
                                D6[:, 3 * RMAX : 3 * RMAX + 3 * fr].rearrange(
                                    "p (c z x) -> p c z x", c=3, z=zr, x=xr
                                ),
                                SV0[:, 3:6, 6 * h : 6 * h + zr,
                                    2 + xlo : 2 + xlo + xr],
                                SV[:, 3:6, 2 - sz : 2 - sz + zr,
                                   2 + xlo - sx : 2 + xlo - sx + xr],
                                Alu.subtract,
                            )
                            force_core(
                                P6,
                                D6[:, 0 : 3 * fr].rearrange(
                                    "p (c z x) -> p c z x", c=3, z=zr, x=xr
                                ),
                                None, zr, xr, None,
                            )
                            pe_pass(P6, 0, 0, -xlo, zr, xr, fr, False)
                            pe_pass(P6, 1 + (sy + 2), sz, sx - xlo, zr, xr, fr,
                                    False)

                    # phantom correction for the 32 dropped shifts (this half)
                    fr = HOWN
                    nemv = NEM[:, h * HOWN : (h + 1) * HOWN]
                    P6 = p6p.tile([G, 6 * RMAX], f32, tag="p6")
                    nc.vector.tensor_copy(
                        D6[:, 0 : 3 * fr].rearrange(
                            "p (c z x) -> p c z x", c=3, z=HZ, x=G
                        ),
                        SV0[:, 0:3, 6 * h : 6 * h + HZ, 2 : 2 + G],
                    )
                    nc.vector.tensor_copy(
                        D6[:, 3 * RMAX : 3 * RMAX + 3 * fr].rearrange(
                            "p (c z x) -> p c z x", c=3, z=HZ, x=G
                        ),
                        SV0[:, 3:6, 6 * h : 6 * h + HZ, 2 : 2 + G],
                    )
                    force_core(
                        P6,
                        D6[:, 0 : 3 * fr].rearrange(
                            "p (c z x) -> p c z x", c=3, z=HZ, x=G
                        ),
                        None, HZ, G, nemv,
                    )
                    pe_pass(P6, 0, 0, 0, HZ, G, fr, True)

                    # evacuate PSUM -> SBUF -> DRAM (channel c, half h)
                    nc.vector.tensor_copy(OUTF[:, :], PSA[:, :])
                    for c in range(6):
                        nc.sync.dma_start(
                            out_p[:, c * OWN + h * HOWN : c * OWN + (h + 1) * HOWN],
                            OUTF[:, c * HOWN : (c + 1) * HOWN],
                        )

    nc.finalize()
    return nc


def _host_prep(inputs):
    d = float(np.asarray(inputs["d"]))
    x = np.asarray(inputs["compressed_x_grid"], np.float32)
    y = np.asarray(inputs["compressed_y_grid"], np.float32)
    z = np.asarray(inputs["compressed_z_grid"], np.float32)
    vx = np.asarray(inputs["compressed_vx_grid"], np.float32)
    vy = np.asarray(inputs["compressed_vy_grid"], np.float32)
    vz = np.asarray(inputs["compressed_vz_grid"], np.float32)

    cx = np.round(x / np.float32(d)).astype(np.int32)
    cy = np.round(y / np.float32(d)).astype(np.int32)
    cz = np.round(z / np.float32(d)).astype(np.int32)

    grids = np.zeros((6, G, G, G), np.float32)
    for i, v in enumerate((x, y, z, vx, vy, vz)):
        grids[i, cz, cy, cx] = v
    occ = np.zeros((G, G, G), np.float32)
    occ[cz, cy, cx] = 1.0

    _, dropped = _shift_sets()
    nocc = np.zeros((G, G, G), np.float32)
    for s in dropped:
        nocc += np.roll(occ, s, axis=(0, 1, 2))
    nem = np.float32(len(dropped)) - nocc

    rots = np.zeros((G, 6 * G), np.float32)
    rots[np.arange(G), np.arange(G)] = 1.0  # +I
    for i, sy in enumerate((-2, -1, 0, 1, 2)):
        rots[(np.arange(G) + sy) % G, (i + 1) * G + np.arange(G)] = -1.0

    ys = np.arange(-2, G + 2) % G
    xs = np.arange(-2, G + 2) % G
    in_maps = []
    for k in range(NCORES):
        z0 = k * ZP
        zs = np.arange(z0 - 2, z0 + ZP + 2) % G
        ext = grids[:, zs][:, :, ys][:, :, :, xs]  # (6,16,100,100)
        ext = np.ascontiguousarray(ext.transpose(2, 0, 1, 3)).reshape(YE, FREE_E)
        nemk = np.ascontiguousarray(
            nem[z0 : z0 + ZP].transpose(1, 0, 2)
        ).reshape(G, OWN)
        in_maps.append({"ext": ext, "nem": nemk, "rots": rots})
    return in_maps, (cz, cy, cx)


def kernel(**inputs):
    from concourse.bass_utils import run_bass_kernel_spmd

    d = float(np.asarray(inputs["d"]))
    kn = float(np.asarray(inputs["kn"]))
    eta = float(np.asarray(inputs["damping_coefficient_Eta"]))

    in_maps, (cz, cy, cx) = _host_prep(inputs)

    key = (d, kn, eta)
    if key not in _CACHE:
        _CACHE[key] = _build(d, kn, eta)
    nc = _CACHE[key]

    res = run_bass_kernel_spmd(nc, in_maps, core_ids=list(range(NCORES)))
    full = np.empty((9, G, G, G), np.float32)
    for k in range(NCORES):
        o = np.asarray(res.results[k]["out"], np.float32).reshape(G, 9, ZP, G)
        full[:, k * ZP : (k + 1) * ZP] = o.transpose(1, 2, 0, 3)
    return full[:, cz, cy, cx]


# revision 17
# speedup vs baseline: 2.1347x; 1.0004x over previous
"""AI4DEM 5^3-stencil DEM force kernel for 8 TRN2 NeuronCores.

v4: Newton's-third-law pair formulation + TensorEngine accumulation.

  - Host: scatter particles into dense 96^3 grids (one per cell), shard
    along Z into 8 slabs.  Per core DRAM inputs: "ext" (y_ext=100 rows x
    6ch x 16 z_ext x 100 x_ext, halos wrap), "nem" (owned empty-neighbor
    counts for the 32 dropped shifts), "rots" (identity + negated
    y-rotation matrices for the TensorEngine scatter pass).
  - Device (SPMD): engine APs cannot start at arbitrary partitions, so the
    y component of each stencil shift is realized by DMA-staging y-rotated
    slabs (DMA maps DRAM rows onto partitions 0..96).  The 92 active
    shifts are processed as 46 +/-s pairs: the pair's shared quantities
    (diffs, dist, 1/dist, vn, A, B, products P6 = [A,B] x [dx,dy,dz]) are
    computed once on a z/x-extended region; the force field then gets
    P6 at the cell (pass 1, identity weights) and -P6 at the shifted cell
    (pass 2, negated y-rotation weights) via TensorEngine matmuls
    accumulating in PSUM (start=False onto a DVE-zeroed bank region).
    Owned z is processed in two halves of 6 planes so the 6-channel force
    accumulator (6x6x96 fp32) fits in PSUM.
  - The 33 remaining shifts (self + (2,2,1)/(2,2,2) families) can never
    produce overlap between real particles; their only effect is the
    reference's "phantom" interaction with empty cells (gathered zeros),
    corrected exactly with the host-precomputed n_empty channel.
  - Host: gather the 9 dense output grids at the particle cells.
"""

import numpy as np

G = 96
N = 400000
NCORES = 8
ZP = G // NCORES          # 12 owned z-planes per core
HZ = ZP // 2              # 6-plane half slabs
ZE = ZP + 4               # 16 extended z-planes (DRAM)
SZE = 10                  # staged z window per half
YE = G + 4                # 100 extended y rows (DRAM only)
XE = G + 4                # 100 extended x
OWN = ZP * G              # 1152
HOWN = HZ * G             # 576
FREE_S = 6 * SZE * XE     # 6000   staged slab free size
FREE_S0 = 6 * 14 * XE     # 8400   center slab free size (z_ext 2..16)
FREE_E = 6 * ZE * XE      # 9600   DRAM ext row size
RMAX = (HZ + 2) * (G + 2) # 784    max region elems per channel
EPS2 = 1e-8

_CACHE = {}


def _shift_sets():
    active, dropped = [], []
    for sz in range(-2, 3):
        for sy in range(-2, 3):
            for sx in range(-2, 3):
                if (sz, sy, sx) == (0, 0, 0):
                    continue
                m = sorted((abs(sz), abs(sy), abs(sx)))
                if m in ([1, 2, 2], [2, 2, 2]):
                    dropped.append((sz, sy, sx))
                else:
                    active.append((sz, sy, sx))
    assert len(active) == 92 and len(dropped) == 32
    return active, dropped


def _pair_sets():
    """Canonical half of the active shifts: one representative per +/-s pair."""
    active, _ = _shift_sets()
    pairs = [
        s
        for s in active
        if (s[0] > 0)
        or (s[0] == 0 and s[1] > 0)
        or (s[0] == 0 and s[1] == 0 and s[2] > 0)
    ]
    assert len(pairs) == 46
    by_sy = {sy: [p for p in pairs if p[1] == sy] for sy in range(-2, 3)}
    return pairs, by_sy


def _register_custom_ops():
    import concourse.dve_ops as dve_ops_mod
    from concourse.dve_ops import DveOp, OPS, get_dve_sub_opcode, has_src1
    from concourse.dve_spec import Spec, Src0, Src1, sq, lower
    from concourse.dve_uop import DveOpSpec

    def reg(name, spec):
        for op in OPS:
            if op.name == name:
                return op
        tmp = DveOp(name, spec, subdim=False, uops_sha={})
        OPS.append(tmp)
        dve_ops_mod._SUB_OPCODE_FOR_NAME[name] = (
            dve_ops_mod._CUSTOM_DVE_ROW_BASE + len(OPS) - 1
        )
        dve_ops_mod.CUSTOM_DVE_SPECS[name] = spec
        shas = {}
        for ver in ("v3", "v4"):
            try:
                ds = DveOpSpec(
                    name=name,
                    opcode=get_dve_sub_opcode(name),
                    uops=lower(spec, ver=ver),
                    rd1_en=has_src1(spec),
                )
                shas[ver] = ds.sha(ver)
            except Exception:
                pass
        final = DveOp(name, spec, subdim=False, uops_sha=shas)
        for i, op in enumerate(OPS):
            if op.name == name:
                OPS[i] = final
                break
        return final

    sqadd = reg(
        "ANT_SQADD",
        Spec(
            body=sq(Src0) + Src1,
            reference=lambda in0, in1, s0, s1, imm2: in0 * in0 + in1,
        ),
    )
    return sqadd


def _build(d, kn, eta):
    import concourse.mybir as mybir
    from concourse.bacc import Bacc
    from concourse.tile import TileContext

    SQADD = _register_custom_ops()
    f32 = mybir.dt.float32
    Alu = mybir.AluOpType
    Act = mybir.ActivationFunctionType
    pairs, by_sy = _pair_sets()

    # PSUM force-accumulator chunks (bank crossing verified OK on HW)
    chunks = []
    for ch in range(6):
        base = ch * HOWN
        chunks.append((base, base + 5 * G, ch, 0, 5, 0, G))
        chunks.append((base + 5 * G, base + HOWN, ch, 5, 6, 0, G))

    nc = Bacc()
    ext_p = nc.declare_dram_parameter("ext", [YE, FREE_E], f32, isOutput=False)
    nem_p = nc.declare_dram_parameter("nem", [G, OWN], f32, isOutput=False)
    rot_p = nc.declare_dram_parameter("rots", [G, 6 * G], f32, isOutput=False)
    out_p = nc.declare_dram_parameter("out", [G, 9 * OWN], f32, isOutput=True)

    with TileContext(nc) as tc:
        with tc.tile_pool(name="persist", bufs=1) as pp:
            S0 = pp.tile([G, FREE_S0], f32, tag="s0")
            NEM = pp.tile([G, OWN], f32, tag="nem")
            ROTS = pp.tile([G, 6 * G], f32, tag="rots")
            OUTF = pp.tile([G, 3456], f32, tag="outf")

            # center slab: ext rows [2, 98), z_ext [2, 16)
            for c in range(6):
                nc.sync.dma_start(
                    S0[:, c * 14 * XE : (c + 1) * 14 * XE],
                    ext_p[2 : 2 + G, c * ZE * XE + 2 * XE : c * ZE * XE + 16 * XE],
                )
            nc.sync.dma_start(NEM[:, :], nem_p[:, :])
            nc.sync.dma_start(ROTS[:, :], rot_p[:, :])

            SV0 = S0[:, :].rearrange("p (c z x) -> p c z x", c=6, z=14, x=XE)

            def rot(i):  # 0 = +I ; 1.. = -rot(sy=-2..2)
                return ROTS[:, i * G : (i + 1) * G]

            # ---- wall forces -> out channels 6..8 (computed once)
            with tc.tile_pool(name="wall", bufs=1) as wpool:
                W1 = wpool.tile([G, OWN], f32, tag="w1")
                W2 = wpool.tile([G, OWN], f32, tag="w2")
                WO = wpool.tile([G, 3 * OWN], f32, tag="wo")
                WC = wpool.tile([G, 2], f32, tag="wc")
                dsz = G * d
                nc.vector.memset(WC[:, 0:1], kn * d)
                nc.vector.memset(WC[:, 1:2], -kn * (dsz - 2.0 * d))

                def vzx(ap):
                    return ap.rearrange("p (z x) -> p z x", z=ZP, x=G)

                for q in range(3):
                    pv = SV0[:, q, 0:ZP, 2 : 2 + G]
                    och = vzx(WO[:, q * OWN : (q + 1) * OWN])
                    nc.scalar.activation(
                        vzx(W1[:, :]), pv, Act.Relu, bias=WC[:, 0:1], scale=-kn
                    )
                    nc.vector.scalar_tensor_tensor(
                        vzx(W2[:, :]), pv, 0.0, vzx(W1[:, :]), Alu.is_equal, Alu.mult
                    )
                    nc.vector.tensor_sub(och, vzx(W1[:, :]), vzx(W2[:, :]))
                    nc.scalar.activation(
                        vzx(W1[:, :]), pv, Act.Relu, bias=WC[:, 1:2], scale=kn
                    )
                    nc.vector.scalar_tensor_tensor(
                        och, vzx(W1[:, :]), -1.0, och, Alu.mult, Alu.add
                    )
                for j in range(3):
                    nc.sync.dma_start(
                        out_p[:, (6 + j) * OWN : (7 + j) * OWN],
                        WO[:, j * OWN : (j + 1) * OWN],
                    )

            with (
                tc.tile_pool(name="work", bufs=1) as wp,
                tc.tile_pool(name="stage", bufs=2) as sp,
                tc.tile_pool(name="p6pool", bufs=2) as p6p,
                tc.tile_pool(name="psum", bufs=1, space="PSUM") as psp,
            ):
                PSA = psp.tile([G, 3456], f32, tag="psa")
                D6 = wp.tile([G, 6 * RMAX], f32, tag="d6")
                M3 = wp.tile([G, 3 * RMAX], f32, tag="m3")
                S2A = wp.tile([G, RMAX], f32, tag="s2a")
                DIST = wp.tile([G, RMAX], f32, tag="dist")
                INV = wp.tile([G, RMAX], f32, tag="inv")
                AT = wp.tile([G, RMAX], f32, tag="at")
                VNA = wp.tile([G, RMAX], f32, tag="vna")
                INV2 = wp.tile([G, RMAX], f32, tag="inv2")
                AB = wp.tile([G, 2 * RMAX], f32, tag="ab")
                CONST = wp.tile([G, 2], f32, tag="const")

                nc.vector.memset(CONST[:, 0:1], EPS2)
                nc.vector.memset(CONST[:, 1:2], 2.0 * d * kn)

                def force_core(P6, d03v, d36v, zr, xr, ab_scale_ap):
                    """Given diff views (3,zr,xr) living in D6, produce A, B
                    and the products P6 = [A,B] x [dx,dy,dz] on the region."""
                    fr = zr * xr

                    def t2(t):
                        return t[:, 0:fr].rearrange("p (z x) -> p z x", z=zr, x=xr)

                    nc.scalar.activation(t2(S2A), d03v[:, 0], Act.Square)
                    # vn numerator first: fills the ACT-latency bubble on DVE
                    nc.vector.tensor_tensor(
                        M3[:, 0 : 3 * fr], D6[:, 0 : 3 * fr],
                        D6[:, 3 * RMAX : 3 * RMAX + 3 * fr], Alu.mult,
                    )
                    nc.vector.tensor_add(
                        VNA[:, 0:fr], M3[:, 0:fr], M3[:, fr : 2 * fr]
                    )
                    nc.vector.tensor_add(
                        VNA[:, 0:fr], VNA[:, 0:fr], M3[:, 2 * fr : 3 * fr]
                    )
                    nc.vector._custom_dve(
                        SQADD, out=t2(S2A), in0=d03v[:, 1], in1=t2(S2A)
                    )
                    nc.vector._custom_dve(
                        SQADD, out=t2(S2A), in0=d03v[:, 2], in1=t2(S2A)
                    )
                    nc.scalar.activation(
                        DIST[:, 0:fr], S2A[:, 0:fr], Act.Sqrt, bias=CONST[:, 0:1]
                    )
                    nc.vector.reciprocal_approx_fast(
                        out=INV[:, 0:fr], in_=DIST[:, 0:fr]
                    )
                    nc.scalar.activation(
                        AT[:, 0:fr], DIST[:, 0:fr], Act.Relu,
                        bias=CONST[:, 1:2], scale=-kn,
                    )
                    # A = -AT*inv ; B = eta*vn*mask*inv^2
                    nc.vector.scalar_tensor_tensor(
                        AB[:, 0:fr], AT[:, 0:fr], -1.0, INV[:, 0:fr],
                        Alu.mult, Alu.mult,
                    )
                    nc.scalar.activation(INV2[:, 0:fr], INV[:, 0:fr], Act.Square)
                    nc.vector.scalar_tensor_tensor(
                        INV2[:, 0:fr], AT[:, 0:fr], 0.0, INV2[:, 0:fr],
                        Alu.is_gt, Alu.mult,
                    )
                    nc.vector.scalar_tensor_tensor(
                        AB[:, RMAX : RMAX + fr], VNA[:, 0:fr], eta, INV2[:, 0:fr],
                        Alu.mult, Alu.mult,
                    )
                    if ab_scale_ap is not None:
                        nc.vector.tensor_tensor(
                            AB[:, 0:fr], AB[:, 0:fr], ab_scale_ap, Alu.mult
                        )
                        nc.vector.tensor_tensor(
                            AB[:, RMAX : RMAX + fr], AB[:, RMAX : RMAX + fr],
                            ab_scale_ap, Alu.mult,
                        )
                    # all 6 products in one op via zero-stride broadcast dims
                    abv = AB[:, :].rearrange(
                        "p (a b k) -> p a b k", a=2, b=1, k=RMAX
                    )[:, :, :, 0:fr]
                    lst = abv.ap
                    lst[2] = [0, 3]
                    abv.ap = lst
                    d6v = D6[:, 0 : 3 * fr].rearrange(
                        "p (a q k) -> p a q k", a=1, q=3, k=fr
                    )
                    lst = d6v.ap
                    lst[1] = [0, 2]
                    d6v.ap = lst
                    p6v = P6[:, 0 : 6 * fr].rearrange(
                        "p (a q k) -> p a q k", a=2, q=3, k=fr
                    )
                    nc.vector.tensor_tensor(p6v, abv, d6v, Alu.mult)
                    return fr

                def pe_pass(P6, rot_idx, zoff, xoff, zr, xr, fr, stop):
                    """PSA[(ch, z, x)] += rot . P6[(ch, z+zoff, x+xoff)]"""
                    P6v = P6[:, 0 : 6 * fr].rearrange(
                        "p (c z x) -> p c z x", c=6, z=zr, x=xr
                    )
                    for k, (o0, o1, ch, z0, z1, x0, x1) in enumerate(chunks):
                        nc.tensor.matmul(
                            PSA[:, o0:o1],
                            rot(rot_idx),
                            P6v[:, ch, z0 + zoff : z1 + zoff, x0 + xoff : x1 + xoff],
                            start=False,
                            stop=stop and k == len(chunks) - 1,
                            skip_group_check=True,
                        )

                for h in range(2):
                    nc.vector.memset(PSA[:, :], 0.0)
                    for sy in (-2, -1, 0, 1, 2):
                        if not by_sy[sy]:
                            continue
                        S = sp.tile([G, FREE_S], f32, tag="sst")
                        # staged neighbor slab: rows y = p - sy, z_ext window
                        # [6h, 6h+10) of the DRAM ext slab
                        for c in range(6):
                            nc.sync.dma_start(
                                S[:, c * SZE * XE : (c + 1) * SZE * XE],
                                ext_p[
                                    2 - sy : 2 - sy + G,
                                    c * ZE * XE
                                    + 6 * h * XE : c * ZE * XE
                                    + (6 * h + SZE) * XE,
                                ],
                            )
                        SV = S[:, :].rearrange(
                            "p (c z x) -> p c z x", c=6, z=SZE, x=XE
                        )
                        for sz, _sy, sx in by_sy[sy]:
                            zr = HZ + sz
                            xr = G + abs(sx)
                            xlo = min(sx, 0)
                            fr = zr * xr
                            P6 = p6p.tile([G, 6 * RMAX], f32, tag="p6")

                            nc.vector.tensor_tensor(
                                D6[:, 0 : 3 * fr].rearrange(
                                    "p (c z x) -> p c z x", c=3, z=zr, x=xr
                                ),
                                SV0[:, 0:3, 6 * h : 6 * h + zr,
                                    2 + xlo : 2 + xlo + xr],
                                SV[:, 0:3, 2 - sz : 2 - sz + zr,
                                   2 + xlo - sx : 2 + xlo - sx + xr],
                                Alu.subtract,
                            )
                            nc.vector.tensor_tensor(
                                D6[:, 3 * RMAX : 3 * RMAX + 3 * fr].rearrange(
                                    "p (c z x) -> p c z x", c=3, z=zr, x=xr
                                ),
                                SV0[:, 3:6, 6 * h : 6 * h + zr,
                                    2 + xlo : 2 + xlo + xr],
                                SV[:, 3:6, 2 - sz : 2 - sz + zr,
                                   2 + xlo - sx : 2 + xlo - sx + xr],
                                Alu.subtract,
                            )
                            force_core(
                                P6,
                                D6[:, 0 : 3 * fr].rearrange(
                                    "p (c z x) -> p c z x", c=3, z=zr, x=xr
                                ),
                                None, zr, xr, None,
                            )
                            pe_pass(P6, 0, 0, -xlo, zr, xr, fr, False)
                            pe_pass(P6, 1 + (sy + 2), sz, sx - xlo, zr, xr, fr,
                                    False)

                    # phantom correction for the 32 dropped shifts (this half)
                    fr = HOWN
                    nemv = NEM[:, h * HOWN : (h + 1) * HOWN]
                    P6 = p6p.tile([G, 6 * RMAX], f32, tag="p6")
                    nc.vector.tensor_copy(
                        D6[:, 0 : 3 * fr].rearrange(
                            "p (c z x) -> p c z x", c=3, z=HZ, x=G
                        ),
                        SV0[:, 0:3, 6 * h : 6 * h + HZ, 2 : 2 + G],
                    )
                    nc.vector.tensor_copy(
                        D6[:, 3 * RMAX : 3 * RMAX + 3 * fr].rearrange(
                            "p (c z x) -> p c z x", c=3, z=HZ, x=G
                        ),
                        SV0[:, 3:6, 6 * h : 6 * h + HZ, 2 : 2 + G],
                    )
                    force_core(
                        P6,
                        D6[:, 0 : 3 * fr].rearrange(
                            "p (c z x) -> p c z x", c=3, z=HZ, x=G
                        ),
                        None, HZ, G, nemv,
                    )
                    pe_pass(P6, 0, 0, 0, HZ, G, fr, True)

                    # evacuate PSUM -> SBUF -> DRAM (channel c, half h)
                    nc.vector.tensor_copy(OUTF[:, :], PSA[:, :])
                    for c in range(6):
                        nc.sync.dma_start(
                            out_p[:, c * OWN + h * HOWN : c * OWN + (h + 1) * HOWN],
                            OUTF[:, c * HOWN : (c + 1) * HOWN],
                        )

    nc.finalize()
    return nc


def _host_prep(inputs):
    d = float(np.asarray(inputs["d"]))
    x = np.asarray(inputs["compressed_x_grid"], np.float32)
    y = np.asarray(inputs["compressed_y_grid"], np.float32)
    z = np.asarray(inputs["compressed_z_grid"], np.float32)
    vx = np.asarray(inputs["compressed_vx_grid"], np.float32)
    vy = np.asarray(inputs["compressed_vy_grid"], np.float32)
    vz = np.asarray(inputs["compressed_vz_grid"], np.float32)

    cx = np.round(x / np.float32(d)).astype(np.int32)
    cy = np.round(y / np.float32(d)).astype(np.int32)
    cz = np.round(z / np.float32(d)).astype(np.int32)

    grids = np.zeros((6, G, G, G), np.float32)
    for i, v in enumerate((x, y, z, vx, vy, vz)):
        grids[i, cz, cy, cx] = v
    occ = np.zeros((G, G, G), np.float32)
    occ[cz, cy, cx] = 1.0

    _, dropped = _shift_sets()
    nocc = np.zeros((G, G, G), np.float32)
    for s in dropped:
        nocc += np.roll(occ, s, axis=(0, 1, 2))
    nem = np.float32(len(dropped)) - nocc

    rots = np.zeros((G, 6 * G), np.float32)
    rots[np.arange(G), np.arange(G)] = 1.0  # +I
    for i, sy in enumerate((-2, -1, 0, 1, 2)):
        rots[(np.arange(G) + sy) % G, (i + 1) * G + np.arange(G)] = -1.0

    ys = np.arange(-2, G + 2) % G
    xs = np.arange(-2, G + 2) % G
    in_maps = []
    for k in range(NCORES):
        z0 = k * ZP
        zs = np.arange(z0 - 2, z0 + ZP + 2) % G
        ext = grids[:, zs][:, :, ys][:, :, :, xs]  # (6,16,100,100)
        ext = np.ascontiguousarray(ext.transpose(2, 0, 1, 3)).reshape(YE, FREE_E)
        nemk = np.ascontiguousarray(
            nem[z0 : z0 + ZP].transpose(1, 0, 2)
        ).reshape(G, OWN)
        in_maps.append({"ext": ext, "nem": nemk, "rots": rots})
    return in_maps, (cz, cy, cx)


def kernel(**inputs):
    from concourse.bass_utils import run_bass_kernel_spmd

    d = float(np.asarray(inputs["d"]))
    kn = float(np.asarray(inputs["kn"]))
    eta = float(np.asarray(inputs["damping_coefficient_Eta"]))

    in_maps, (cz, cy, cx) = _host_prep(inputs)

    key = (d, kn, eta)
    if key not in _CACHE:
        _CACHE[key] = _build(d, kn, eta)
    nc = _CACHE[key]

    res = run_bass_kernel_spmd(nc, in_maps, core_ids=list(range(NCORES)))
    full = np.empty((9, G, G, G), np.float32)
    for k in range(NCORES):
        o = np.asarray(res.results[k]["out"], np.float32).reshape(G, 9, ZP, G)
        full[:, k * ZP : (k + 1) * ZP] = o.transpose(1, 2, 0, 3)
    return full[:, cz, cy, cx]


# revision 20
# speedup vs baseline: 2.1514x; 1.0078x over previous
"""AI4DEM 5^3-stencil DEM force kernel for 8 TRN2 NeuronCores.

v4: Newton's-third-law pair formulation + TensorEngine accumulation.

  - Host: scatter particles into dense 96^3 grids (one per cell), shard
    along Z into 8 slabs.  Per core DRAM inputs: "ext" (y_ext=100 rows x
    6ch x 16 z_ext x 100 x_ext, halos wrap), "nem" (owned empty-neighbor
    counts for the 32 dropped shifts), "rots" (identity + negated
    y-rotation matrices for the TensorEngine scatter pass).
  - Device (SPMD): engine APs cannot start at arbitrary partitions, so the
    y component of each stencil shift is realized by DMA-staging y-rotated
    slabs (DMA maps DRAM rows onto partitions 0..96).  The 92 active
    shifts are processed as 46 +/-s pairs: the pair's shared quantities
    (diffs, dist, 1/dist, vn, A, B, products P6 = [A,B] x [dx,dy,dz]) are
    computed once on a z/x-extended region; the force field then gets
    P6 at the cell (pass 1, identity weights) and -P6 at the shifted cell
    (pass 2, negated y-rotation weights) via TensorEngine matmuls
    accumulating in PSUM (start=False onto a DVE-zeroed bank region).
    Owned z is processed in two halves of 6 planes so the 6-channel force
    accumulator (6x6x96 fp32) fits in PSUM.
  - The 33 remaining shifts (self + (2,2,1)/(2,2,2) families) can never
    produce overlap between real particles; their only effect is the
    reference's "phantom" interaction with empty cells (gathered zeros),
    corrected exactly with the host-precomputed n_empty channel.
  - Host: gather the 9 dense output grids at the particle cells.
"""

import numpy as np

G = 96
N = 400000
NCORES = 8
ZP = G // NCORES          # 12 owned z-planes per core
HZ = ZP // 2              # 6-plane half slabs
ZE = ZP + 4               # 16 extended z-planes (DRAM)
SZE = 10                  # staged z window per half
YE = G + 4                # 100 extended y rows (DRAM only)
XE = G + 4                # 100 extended x
OWN = ZP * G              # 1152
HOWN = HZ * G             # 576
FREE_S = 6 * SZE * XE     # 6000   staged slab free size
FREE_S0 = 6 * 14 * XE     # 8400   center slab free size (z_ext 2..16)
FREE_E = 6 * ZE * XE      # 9600   DRAM ext row size
RMAX = (HZ + 2) * (G + 2) # 784    max region elems per channel
EPS2 = 1e-8

_CACHE = {}


def _shift_sets():
    active, dropped = [], []
    for sz in range(-2, 3):
        for sy in range(-2, 3):
            for sx in range(-2, 3):
                if (sz, sy, sx) == (0, 0, 0):
                    continue
                m = sorted((abs(sz), abs(sy), abs(sx)))
                if m in ([1, 2, 2], [2, 2, 2]):
                    dropped.append((sz, sy, sx))
                else:
                    active.append((sz, sy, sx))
    assert len(active) == 92 and len(dropped) == 32
    return active, dropped


def _pair_sets():
    """Canonical half of the active shifts: one representative per +/-s pair."""
    active, _ = _shift_sets()
    pairs = [
        s
        for s in active
        if (s[0] > 0)
        or (s[0] == 0 and s[1] > 0)
        or (s[0] == 0 and s[1] == 0 and s[2] > 0)
    ]
    assert len(pairs) == 46
    by_sy = {sy: [p for p in pairs if p[1] == sy] for sy in range(-2, 3)}
    return pairs, by_sy


def _register_custom_ops():
    import concourse.dve_ops as dve_ops_mod
    from concourse.dve_ops import DveOp, OPS, get_dve_sub_opcode, has_src1
    from concourse.dve_spec import Spec, Src0, Src1, sq, lower
    from concourse.dve_uop import DveOpSpec

    def reg(name, spec):
        for op in OPS:
            if op.name == name:
                return op
        tmp = DveOp(name, spec, subdim=False, uops_sha={})
        OPS.append(tmp)
        dve_ops_mod._SUB_OPCODE_FOR_NAME[name] = (
            dve_ops_mod._CUSTOM_DVE_ROW_BASE + len(OPS) - 1
        )
        dve_ops_mod.CUSTOM_DVE_SPECS[name] = spec
        shas = {}
        for ver in ("v3", "v4"):
            try:
                ds = DveOpSpec(
                    name=name,
                    opcode=get_dve_sub_opcode(name),
                    uops=lower(spec, ver=ver),
                    rd1_en=has_src1(spec),
                )
                shas[ver] = ds.sha(ver)
            except Exception:
                pass
        final = DveOp(name, spec, subdim=False, uops_sha=shas)
        for i, op in enumerate(OPS):
            if op.name == name:
                OPS[i] = final
                break
        return final

    sqadd = reg(
        "ANT_SQADD",
        Spec(
            body=sq(Src0) + Src1,
            reference=lambda in0, in1, s0, s1, imm2: in0 * in0 + in1,
        ),
    )
    return sqadd


def _build(d, kn, eta):
    import concourse.mybir as mybir
    from concourse.bacc import Bacc
    from concourse.tile import TileContext

    SQADD = _register_custom_ops()
    f32 = mybir.dt.float32
    Alu = mybir.AluOpType
    Act = mybir.ActivationFunctionType
    pairs, by_sy = _pair_sets()

    # PSUM force-accumulator chunks (bank crossing verified OK on HW)
    chunks = []
    for ch in range(6):
        base = ch * HOWN
        chunks.append((base, base + 5 * G, ch, 0, 5, 0, G))
        chunks.append((base + 5 * G, base + HOWN, ch, 5, 6, 0, G))

    nc = Bacc()
    ext_p = nc.declare_dram_parameter("ext", [YE, FREE_E], f32, isOutput=False)
    nem_p = nc.declare_dram_parameter("nem", [G, OWN], f32, isOutput=False)
    rot_p = nc.declare_dram_parameter("rots", [G, 6 * G], f32, isOutput=False)
    out_p = nc.declare_dram_parameter("out", [G, 9 * OWN], f32, isOutput=True)

    with TileContext(nc) as tc:
        with tc.tile_pool(name="persist", bufs=1) as pp:
            S0 = pp.tile([G, FREE_S0], f32, tag="s0")
            NEM = pp.tile([G, OWN], f32, tag="nem")
            ROTS = pp.tile([G, 6 * G], f32, tag="rots")
            OUTF = pp.tile([G, 3456], f32, tag="outf")

            # center slab: ext rows [2, 98), z_ext [2, 16)
            for c in range(6):
                nc.sync.dma_start(
                    S0[:, c * 14 * XE : (c + 1) * 14 * XE],
                    ext_p[2 : 2 + G, c * ZE * XE + 2 * XE : c * ZE * XE + 16 * XE],
                )
            nc.sync.dma_start(NEM[:, :], nem_p[:, :])
            nc.sync.dma_start(ROTS[:, :], rot_p[:, :])

            SV0 = S0[:, :].rearrange("p (c z x) -> p c z x", c=6, z=14, x=XE)

            def rot(i):  # 0 = +I ; 1.. = -rot(sy=-2..2)
                return ROTS[:, i * G : (i + 1) * G]

            # ---- wall forces -> out channels 6..8 (computed once)
            with tc.tile_pool(name="wall", bufs=1) as wpool:
                W1 = wpool.tile([G, OWN], f32, tag="w1")
                W2 = wpool.tile([G, OWN], f32, tag="w2")
                WO = wpool.tile([G, 3 * OWN], f32, tag="wo")
                WC = wpool.tile([G, 2], f32, tag="wc")
                dsz = G * d
                nc.vector.memset(WC[:, 0:1], kn * d)
                nc.vector.memset(WC[:, 1:2], -kn * (dsz - 2.0 * d))

                def vzx(ap):
                    return ap.rearrange("p (z x) -> p z x", z=ZP, x=G)

                for q in range(3):
                    pv = SV0[:, q, 0:ZP, 2 : 2 + G]
                    och = vzx(WO[:, q * OWN : (q + 1) * OWN])
                    nc.scalar.activation(
                        vzx(W1[:, :]), pv, Act.Relu, bias=WC[:, 0:1], scale=-kn
                    )
                    nc.vector.scalar_tensor_tensor(
                        vzx(W2[:, :]), pv, 0.0, vzx(W1[:, :]), Alu.is_equal, Alu.mult
                    )
                    nc.vector.tensor_sub(och, vzx(W1[:, :]), vzx(W2[:, :]))
                    nc.scalar.activation(
                        vzx(W1[:, :]), pv, Act.Relu, bias=WC[:, 1:2], scale=kn
                    )
                    nc.vector.scalar_tensor_tensor(
                        och, vzx(W1[:, :]), -1.0, och, Alu.mult, Alu.add
                    )
                for j in range(3):
                    nc.sync.dma_start(
                        out_p[:, (6 + j) * OWN : (7 + j) * OWN],
                        WO[:, j * OWN : (j + 1) * OWN],
                    )

            with (
                tc.tile_pool(name="work", bufs=1) as wp,
                tc.tile_pool(name="stage", bufs=2) as sp,
                tc.tile_pool(name="p6pool", bufs=2) as p6p,
                tc.tile_pool(name="psum", bufs=1, space="PSUM") as psp,
            ):
                PSA = psp.tile([G, 3456], f32, tag="psa")
                D6 = wp.tile([G, 6 * RMAX], f32, tag="d6")
                M3 = wp.tile([G, 3 * RMAX], f32, tag="m3")
                S2A = wp.tile([G, RMAX], f32, tag="s2a")
                DIST = wp.tile([G, RMAX], f32, tag="dist")
                INV = wp.tile([G, RMAX], f32, tag="inv")
                AT = wp.tile([G, RMAX], f32, tag="at")
                VNA = wp.tile([G, RMAX], f32, tag="vna")
                INV2 = wp.tile([G, RMAX], f32, tag="inv2")
                AB = wp.tile([G, 2 * RMAX], f32, tag="ab")
                CONST = wp.tile([G, 2], f32, tag="const")

                nc.vector.memset(CONST[:, 0:1], EPS2)
                nc.vector.memset(CONST[:, 1:2], 2.0 * d * kn)

                def force_core(P6, d03v, d36v, zr, xr, ab_scale_ap):
                    """Given diff views (3,zr,xr) living in D6, produce A, B
                    and the products P6 = [A,B] x [dx,dy,dz] on the region."""
                    fr = zr * xr

                    def t2(t):
                        return t[:, 0:fr].rearrange("p (z x) -> p z x", z=zr, x=xr)

                    nc.scalar.activation(t2(S2A), d03v[:, 0], Act.Square)
                    # vn numerator first: fills the ACT-latency bubble on DVE
                    nc.vector.tensor_tensor(
                        M3[:, 0 : 3 * fr], D6[:, 0 : 3 * fr],
                        D6[:, 3 * RMAX : 3 * RMAX + 3 * fr], Alu.mult,
                    )
                    nc.vector.tensor_add(
                        VNA[:, 0:fr], M3[:, 0:fr], M3[:, fr : 2 * fr]
                    )
                    nc.vector.tensor_add(
                        VNA[:, 0:fr], VNA[:, 0:fr], M3[:, 2 * fr : 3 * fr]
                    )
                    nc.vector._custom_dve(
                        SQADD, out=t2(S2A), in0=d03v[:, 1], in1=t2(S2A)
                    )
                    nc.vector._custom_dve(
                        SQADD, out=t2(S2A), in0=d03v[:, 2], in1=t2(S2A)
                    )
                    nc.scalar.activation(
                        DIST[:, 0:fr], S2A[:, 0:fr], Act.Sqrt, bias=CONST[:, 0:1]
                    )
                    nc.vector.reciprocal_approx_fast(
                        out=INV[:, 0:fr], in_=DIST[:, 0:fr]
                    )
                    nc.scalar.activation(
                        AT[:, 0:fr], DIST[:, 0:fr], Act.Relu,
                        bias=CONST[:, 1:2], scale=-kn,
                    )
                    # A = -AT*inv ; B = eta*vn*mask*inv^2
                    nc.vector.scalar_tensor_tensor(
                        AB[:, 0:fr], AT[:, 0:fr], -1.0, INV[:, 0:fr],
                        Alu.mult, Alu.mult,
                    )
                    nc.scalar.activation(INV2[:, 0:fr], INV[:, 0:fr], Act.Square)
                    nc.vector.scalar_tensor_tensor(
                        INV2[:, 0:fr], AT[:, 0:fr], 0.0, INV2[:, 0:fr],
                        Alu.is_gt, Alu.mult,
                    )
                    nc.vector.scalar_tensor_tensor(
                        AB[:, RMAX : RMAX + fr], VNA[:, 0:fr], eta, INV2[:, 0:fr],
                        Alu.mult, Alu.mult,
                    )
                    if ab_scale_ap is not None:
                        nc.vector.tensor_tensor(
                            AB[:, 0:fr], AB[:, 0:fr], ab_scale_ap, Alu.mult
                        )
                        nc.vector.tensor_tensor(
                            AB[:, RMAX : RMAX + fr], AB[:, RMAX : RMAX + fr],
                            ab_scale_ap, Alu.mult,
                        )
                    # all 6 products in one op via zero-stride broadcast dims
                    abv = AB[:, :].rearrange(
                        "p (a b k) -> p a b k", a=2, b=1, k=RMAX
                    )[:, :, :, 0:fr]
                    lst = abv.ap
                    lst[2] = [0, 3]
                    abv.ap = lst
                    d6v = D6[:, 0 : 3 * fr].rearrange(
                        "p (a q k) -> p a q k", a=1, q=3, k=fr
                    )
                    lst = d6v.ap
                    lst[1] = [0, 2]
                    d6v.ap = lst
                    p6v = P6[:, 0 : 6 * fr].rearrange(
                        "p (a q k) -> p a q k", a=2, q=3, k=fr
                    )
                    nc.vector.tensor_tensor(p6v, abv, d6v, Alu.mult)
                    return fr

                def pe_pass(P6, rot_idx, zoff, xoff, zr, xr, fr, stop):
                    """PSA[(ch, z, x)] += rot . P6[(ch, z+zoff, x+xoff)]"""
                    P6v = P6[:, 0 : 6 * fr].rearrange(
                        "p (c z x) -> p c z x", c=6, z=zr, x=xr
                    )
                    for k, (o0, o1, ch, z0, z1, x0, x1) in enumerate(chunks):
                        nc.tensor.matmul(
                            PSA[:, o0:o1],
                            rot(rot_idx),
                            P6v[:, ch, z0 + zoff : z1 + zoff, x0 + xoff : x1 + xoff],
                            start=False,
                            stop=stop and k == len(chunks) - 1,
                            skip_group_check=True,
                        )

                for h in range(2):
                    nc.scalar.memzero(PSA[:, :])
                    for sy in (-2, -1, 0, 1, 2):
                        if not by_sy[sy]:
                            continue
                        S = sp.tile([G, FREE_S], f32, tag="sst")
                        # staged neighbor slab: rows y = p - sy, z_ext window
                        # [6h, 6h+10) of the DRAM ext slab
                        for c in range(6):
                            nc.sync.dma_start(
                                S[:, c * SZE * XE : (c + 1) * SZE * XE],
                                ext_p[
                                    2 - sy : 2 - sy + G,
                                    c * ZE * XE
                                    + 6 * h * XE : c * ZE * XE
                                    + (6 * h + SZE) * XE,
                                ],
                            )
                        SV = S[:, :].rearrange(
                            "p (c z x) -> p c z x", c=6, z=SZE, x=XE
                        )
                        for sz, _sy, sx in by_sy[sy]:
                            zr = HZ + sz
                            xr = G + abs(sx)
                            xlo = min(sx, 0)
                            fr = zr * xr
                            P6 = p6p.tile([G, 6 * RMAX], f32, tag="p6")

                            nc.vector.tensor_tensor(
                                D6[:, 0 : 3 * fr].rearrange(
                                    "p (c z x) -> p c z x", c=3, z=zr, x=xr
                                ),
                                SV0[:, 0:3, 6 * h : 6 * h + zr,
                                    2 + xlo : 2 + xlo + xr],
                                SV[:, 0:3, 2 - sz : 2 - sz + zr,
                                   2 + xlo - sx : 2 + xlo - sx + xr],
                                Alu.subtract,
                            )
                            nc.vector.tensor_tensor(
                                D6[:, 3 * RMAX : 3 * RMAX + 3 * fr].rearrange(
                                    "p (c z x) -> p c z x", c=3, z=zr, x=xr
                                ),
                                SV0[:, 3:6, 6 * h : 6 * h + zr,
                                    2 + xlo : 2 + xlo + xr],
                                SV[:, 3:6, 2 - sz : 2 - sz + zr,
                                   2 + xlo - sx : 2 + xlo - sx + xr],
                                Alu.subtract,
                            )
                            force_core(
                                P6,
                                D6[:, 0 : 3 * fr].rearrange(
                                    "p (c z x) -> p c z x", c=3, z=zr, x=xr
                                ),
                                None, zr, xr, None,
                            )
                            pe_pass(P6, 0, 0, -xlo, zr, xr, fr, False)
                            pe_pass(P6, 1 + (sy + 2), sz, sx - xlo, zr, xr, fr,
                                    False)

                    # phantom correction for the 32 dropped shifts (this half)
                    fr = HOWN
                    nemv = NEM[:, h * HOWN : (h + 1) * HOWN]
                    P6 = p6p.tile([G, 6 * RMAX], f32, tag="p6")
                    nc.scalar.copy(
                        D6[:, 0 : 3 * fr].rearrange(
                            "p (c z x) -> p c z x", c=3, z=HZ, x=G
                        ),
                        SV0[:, 0:3, 6 * h : 6 * h + HZ, 2 : 2 + G],
                    )
                    nc.scalar.copy(
                        D6[:, 3 * RMAX : 3 * RMAX + 3 * fr].rearrange(
                            "p (c z x) -> p c z x", c=3, z=HZ, x=G
                        ),
                        SV0[:, 3:6, 6 * h : 6 * h + HZ, 2 : 2 + G],
                    )
                    force_core(
                        P6,
                        D6[:, 0 : 3 * fr].rearrange(
                            "p (c z x) -> p c z x", c=3, z=HZ, x=G
                        ),
                        None, HZ, G, nemv,
                    )
                    pe_pass(P6, 0, 0, 0, HZ, G, fr, True)

                    # evacuate PSUM -> SBUF -> DRAM (channel c, half h)
                    nc.scalar.copy(OUTF[:, :], PSA[:, :])
                    for c in range(6):
                        nc.sync.dma_start(
                            out_p[:, c * OWN + h * HOWN : c * OWN + (h + 1) * HOWN],
                            OUTF[:, c * HOWN : (c + 1) * HOWN],
                        )

    nc.finalize()
    return nc


def _host_prep(inputs):
    d = float(np.asarray(inputs["d"]))
    x = np.asarray(inputs["compressed_x_grid"], np.float32)
    y = np.asarray(inputs["compressed_y_grid"], np.float32)
    z = np.asarray(inputs["compressed_z_grid"], np.float32)
    vx = np.asarray(inputs["compressed_vx_grid"], np.float32)
    vy = np.asarray(inputs["compressed_vy_grid"], np.float32)
    vz = np.asarray(inputs["compressed_vz_grid"], np.float32)

    cx = np.round(x / np.float32(d)).astype(np.int32)
    cy = np.round(y / np.float32(d)).astype(np.int32)
    cz = np.round(z / np.float32(d)).astype(np.int32)

    grids = np.zeros((6, G, G, G), np.float32)
    for i, v in enumerate((x, y, z, vx, vy, vz)):
        grids[i, cz, cy, cx] = v
    occ = np.zeros((G, G, G), np.float32)
    occ[cz, cy, cx] = 1.0

    _, dropped = _shift_sets()
    nocc = np.zeros((G, G, G), np.float32)
    for s in dropped:
        nocc += np.roll(occ, s, axis=(0, 1, 2))
    nem = np.float32(len(dropped)) - nocc

    rots = np.zeros((G, 6 * G), np.float32)
    rots[np.arange(G), np.arange(G)] = 1.0  # +I
    for i, sy in enumerate((-2, -1, 0, 1, 2)):
        rots[(np.arange(G) + sy) % G, (i + 1) * G + np.arange(G)] = -1.0

    ys = np.arange(-2, G + 2) % G
    xs = np.arange(-2, G + 2) % G
    in_maps = []
    for k in range(NCORES):
        z0 = k * ZP
        zs = np.arange(z0 - 2, z0 + ZP + 2) % G
        ext = grids[:, zs][:, :, ys][:, :, :, xs]  # (6,16,100,100)
        ext = np.ascontiguousarray(ext.transpose(2, 0, 1, 3)).reshape(YE, FREE_E)
        nemk = np.ascontiguousarray(
            nem[z0 : z0 + ZP].transpose(1, 0, 2)
        ).reshape(G, OWN)
        in_maps.append({"ext": ext, "nem": nemk, "rots": rots})
    return in_maps, (cz, cy, cx)


def kernel(**inputs):
    from concourse.bass_utils import run_bass_kernel_spmd

    d = float(np.asarray(inputs["d"]))
    kn = float(np.asarray(inputs["kn"]))
    eta = float(np.asarray(inputs["damping_coefficient_Eta"]))

    in_maps, (cz, cy, cx) = _host_prep(inputs)

    key = (d, kn, eta)
    if key not in _CACHE:
        _CACHE[key] = _build(d, kn, eta)
    nc = _CACHE[key]

    res = run_bass_kernel_spmd(nc, in_maps, core_ids=list(range(NCORES)))
    full = np.empty((9, G, G, G), np.float32)
    for k in range(NCORES):
        o = np.asarray(res.results[k]["out"], np.float32).reshape(G, 9, ZP, G)
        full[:, k * ZP : (k + 1) * ZP] = o.transpose(1, 2, 0, 3)
    return full[:, cz, cy, cx]
